# revision 37
# baseline (speedup 1.0000x reference)
"""Trainium2 Bass kernel for a Deformable-DETR style encoder block.

Sharding: 8 NeuronCores = 4 batch samples x 2 query-halves.

Wire-traffic-optimized: each core uploads only its OWN half of the
features/pos/ref (fp16), computes the value projection of that half, and
the full per-batch value table is assembled on-device with a pair
AllGather (cores 2b <-> 2b+1).  All matmul weights ship as fp16 and all
matmuls run in fp16 (PSUM accumulates fp32).  The output returns as fp16.

Per core:
  - value projection over the own half -> pair AllGather -> fp16 "patch
    table" in DRAM: for cell (y,x) and head h the 2x2 neighborhood
    [V[y,x], V[y,x+1], V[y+1,x], V[y+1,x+1]] is packed contiguously
    (4*32 fp16 = 256B), so one dma_gather descriptor fetches a complete
    bilinear patch.
  - offset/attention projections, softmax, bilinear weights and cell
    indices computed query-major (PE transposes feed the matmuls).
  - gpsimd.dma_gather fetches patches; DVE multiplies and tree-reduces.
  - output projection + LayerNorm + FFN + LayerNorm, then DMA out (fp16).
"""

import numpy as np
from contextlib import ExitStack

EMB = 256
NH = 8
NL = 4
NPT = 4
HD = 32
DFFN = 1024
P = 128


def make_cfg(shapes, n_blk_q, grp):
    L = sum(h * w for h, w in shapes)
    starts = np.cumsum([0] + [h * w for h, w in shapes])[:-1].tolist()
    n_blk_full = -(-L // P)
    assert n_blk_q % grp == 0
    return dict(
        shapes=[tuple(s) for s in shapes], starts=starts, L=L,
        LPAD=n_blk_full * P, NBF=n_blk_full, NBQ=n_blk_q, HQ=n_blk_q * P,
        GRP=grp, NGRP=n_blk_q // grp,
    )


CFG_FULL = make_cfg([(100, 100), (50, 50), (25, 25), (13, 13)], 52, 1)
HALF = 6647

# weight blob layout: name -> (element offset, k // P, n), fp16 elements
WORDER = ["W_val", "W_off", "W_attn", "W_out", "W1", "W2"]
WSHAPES = {"W_val": (EMB, EMB), "W_off": (EMB, EMB),
           "W_attn": (EMB, NH * NL * NPT), "W_out": (EMB, EMB),
           "W1": (EMB, DFFN), "W2": (DFFN, EMB)}
WOFFS = {}
_off = 0
for _n in WORDER:
    _k, _c = WSHAPES[_n]
    WOFFS[_n] = (_off, _k // P, _c)
    _off += _k * _c
WTOT = _off
assert WTOT % 8 == 0
WCHUNK = WTOT // 8

# packed small-constant blobs (fp16 / fp32), offsets in elements
SB16ORD = [("b_val", EMB), ("b_off", EMB), ("b_attn", NH * NL * NPT),
           ("b_out", EMB), ("b1", DFFN), ("b2", EMB), ("ones_row", P),
           ("ident", P * P)]
SB16OFF = {}
_off = 0
for _n, _c in SB16ORD:
    SB16OFF[_n] = _off
    _off += _c
SB16TOT = _off
SB32ORD = [("ln1_g", EMB), ("ln1_b", EMB), ("ln2_g", EMB), ("ln2_b", EMB),
           ("cst_xy", 4 * EMB), ("cst_hlp", 3 * P)]
SB32OFF = {}
_off = 0
for _n, _c in SB32ORD:
    SB32OFF[_n] = _off
    _off += _c
SB32TOT = _off


# ------------------------------------------------------- host-side consts ---

def host_constants(cfg):
    shapes, starts = cfg["shapes"], cfg["starts"]
    invnorm = np.zeros(EMB, np.float32)
    pixscale = np.zeros(EMB, np.float32)
    clipmax = np.zeros(EMB, np.float32)
    vmax = np.zeros(EMB, np.float32)
    for h in range(NH):
        for l, (H_, W_) in enumerate(shapes):
            for pt in range(NPT):
                base = h * (NL * NPT * 2) + l * (NPT * 2) + pt * 2
                invnorm[base + 0] = 1.0 / W_
                invnorm[base + 1] = 1.0 / H_
                pixscale[base + 0] = W_
                pixscale[base + 1] = H_
                clipmax[base + 0] = W_ - 2
                clipmax[base + 1] = H_ - 2
                vmax[base + 0] = W_ - 1
                vmax[base + 1] = H_ - 1
    cst_xy = np.stack([invnorm, pixscale, clipmax, vmax])

    wrow = np.zeros(P, np.float32)
    srow = np.zeros(P, np.float32)
    hrow = np.zeros(P, np.float32)
    L = cfg["L"]
    for h in range(NH):
        for l, (H_, W_) in enumerate(shapes):
            for pt in range(NPT):
                base = h * (NL * NPT) + l * NPT + pt
                wrow[base] = W_
                srow[base] = starts[l]
                hrow[base] = h * L
    cst_hlp = np.stack([wrow, srow, hrow])

    ident = np.eye(P, dtype=np.float16)
    ones_row = np.ones((1, P), np.float16)
    return dict(cst_xy=cst_xy, cst_hlp=cst_hlp, ident=ident,
                ones_row=ones_row)


# ------------------------------------------------------------- emission ---

def emit_kernel(tc, outs, ins, cfg):
    import concourse.bass as bass
    from concourse import mybir

    nc = tc.nc
    op = mybir.AluOpType
    act_f = mybir.ActivationFunctionType
    f32, f16 = mybir.dt.float32, mybir.dt.float16
    i32 = mybir.dt.int32
    AX = mybir.AxisListType

    shapes, starts = cfg["shapes"], cfg["starts"]
    L, NBQ, NGRP = (cfg[k] for k in ("L", "NBQ", "NGRP"))

    ctx = ExitStack()

    def dap(handle, offset, dims):
        return bass.AP(tensor=handle, offset=offset,
                       ap=[list(d) for d in dims])

    def sap(ap0, extra_off, dims):
        return bass.AP(tensor=ap0.tensor, offset=ap0.offset + extra_off,
                       ap=[list(d) for d in dims])

    i8 = mybir.dt.int8

    # ---- internal DRAM ----
    val_half = nc.dram_tensor("val_half", [HALF, EMB], f16, kind="Internal")
    val_full = nc.dram_tensor("val_full", [2 * HALF, EMB], f16,
                              kind="Internal")
    tableT = nc.dram_tensor("tableT", [NH * L, 4 * HD], f16, kind="Internal")
    wb_in = nc.dram_tensor("wb_in", [WCHUNK], f16, kind="Internal")
    wblob = nc.dram_tensor("wblob", [8 * WCHUNK], f16, kind="Internal")

    # ---- pools ----
    cpool = ctx.enter_context(tc.tile_pool(name="consts", bufs=1))
    apool = ctx.enter_context(tc.tile_pool(name="acts", bufs=3))
    wpool = ctx.enter_context(tc.tile_pool(name="wmath", bufs=1))
    gpool = ctx.enter_context(tc.tile_pool(name="gath", bufs=2))
    kpool = ctx.enter_context(tc.tile_pool(name="comb", bufs=2))
    opool = ctx.enter_context(tc.tile_pool(name="outp", bufs=2))
    ps_tr = ctx.enter_context(tc.tile_pool(name="ps_tr", bufs=2, space="PSUM"))
    ps_mm = ctx.enter_context(tc.tile_pool(name="ps_mm", bufs=2, space="PSUM"))
    ps_sm = ctx.enter_context(tc.tile_pool(name="ps_sm", bufs=2, space="PSUM"))

    def dma(out_ap, in_ap):
        nc.sync.dma_start(out=out_ap, in_=in_ap)

    # ---- weights: sharded upload, 8-core AllGather, then load from blob ----
    dma(wb_in.ap()[:], ins["wchunk"][:])
    nc.gpsimd.collective_compute(
        "AllGather",
        mybir.AluOpType.bypass,
        replica_groups=[[0, 1, 2, 3, 4, 5, 6, 7]],
        ins=[wb_in.ap()[:]],
        outs=[wblob.ap()[:]],
    )

    def load_w(name):
        base, a, n = WOFFS[name]
        t = cpool.tile([P, a, n], f16, name=f"s_{name}")
        dma(t, dap(wblob, base, [[n, P], [P * n, a], [1, n]]))
        return t

    Wval = load_w("W_val")
    Woff = load_w("W_off")
    Watt = load_w("W_attn")
    Wout = load_w("W_out")
    W1 = load_w("W1")
    W2 = load_w("W2")

    sb16t = ins["sb16"].tensor
    sb32t = ins["sb32"].tensor

    def load_row(name, n):
        t = cpool.tile([1, n], f16, name=f"r_{name}")
        dma(t, dap(sb16t, SB16OFF[name], [[n, 1], [1, n]]))
        return t

    bval = load_row("b_val", EMB)
    boff = load_row("b_off", EMB)
    batt = load_row("b_attn", NH * NL * NPT)
    bout = load_row("b_out", EMB)
    b1r = load_row("b1", DFFN)
    b2r = load_row("b2", EMB)
    onesr = load_row("ones_row", P)

    def load_bc(off, n, name):
        t = cpool.tile([P, n], f32, name=f"b_{name}")
        dma(t, dap(sb32t, off, [[0, P], [1, n]]))
        return t

    ln1g = load_bc(SB32OFF["ln1_g"], EMB, "ln1g")
    ln1b = load_bc(SB32OFF["ln1_b"], EMB, "ln1b")
    ln2g = load_bc(SB32OFF["ln2_g"], EMB, "ln2g")
    ln2b = load_bc(SB32OFF["ln2_b"], EMB, "ln2b")
    c_invn = load_bc(SB32OFF["cst_xy"], EMB, "invn")
    c_pixs = load_bc(SB32OFF["cst_xy"] + EMB, EMB, "pixs")
    c_clip = load_bc(SB32OFF["cst_xy"] + 2 * EMB, EMB, "clip")
    c_vmax = load_bc(SB32OFF["cst_xy"] + 3 * EMB, EMB, "vmax")
    c_W = load_bc(SB32OFF["cst_hlp"], P, "cw")
    c_S = load_bc(SB32OFF["cst_hlp"] + P, P, "cs")
    c_HL = load_bc(SB32OFF["cst_hlp"] + 2 * P, P, "chl")

    idf16 = cpool.tile([P, P], f16, name="idf16")
    dma(idf16, dap(sb16t, SB16OFF["ident"], [[P, P], [1, P]]))
    eps_t = cpool.tile([P, 1], f32, name="eps_t")
    nc.vector.memset(eps_t[:, :], 1e-5)

    refr = cpool.tile([P, NBQ, 2 * NL], f16, name="refr")
    dma(refr, ins["ref_q"].rearrange("(b p) l c -> p b (l c)", p=P))
    rsct = cpool.tile([P, NBQ, 2], f32, name="rsct")
    dma(rsct, ins["rsc"].rearrange("(b p) c -> p b c", p=P))
    m8sall = cpool.tile([P, NBQ], f32, name="m8sall")
    nc.scalar.mul(m8sall[:, :], rsct[:, :, 1], -8.0)

    def mm(psum_ap, pairs, bias=None):
        seq = list(pairs)
        if bias is not None:
            seq.append((onesr[:1, :psum_ap.shape[0]], bias))
        for i, (lt, rt) in enumerate(seq):
            nc.tensor.matmul(psum_ap, lt, rt,
                             start=(i == 0), stop=(i == len(seq) - 1))

    # ============ P1: value projection of the own half ============
    for blk in range(NBQ):
        fi8 = apool.tile([P, EMB], i8, name="fi8", tag="fi8")
        dma(fi8, ins["feat_h"][blk * P:(blk + 1) * P, :])
        fv = apool.tile([P, EMB], f16, name="fv", tag="fv")
        nc.vector.tensor_scalar_mul(fv[:, :], fi8[:, :],
                                    rsct[:, blk, 0:1])
        ftp = ps_tr.tile([P, 2, P], f16, name="ftp", tag="tr")
        nc.tensor.transpose(ftp[:, 0, :], fv[:, 0:P], idf16[:, :])
        nc.tensor.transpose(ftp[:, 1, :], fv[:, P:EMB], idf16[:, :])
        fts = apool.tile([P, 2, P], f16, name="fts", tag="fts")
        nc.vector.tensor_copy(fts[:, :, :], ftp[:, :, :])
        vp = ps_mm.tile([P, EMB], f32, name="vp", tag="mm")
        mm(vp, [(fts[:, 0, :], Wval[:, 0, :]), (fts[:, 1, :], Wval[:, 1, :])],
           bias=bval[:1, :])
        vf = apool.tile([P, EMB], f16, name="vf", tag="vf")
        nc.vector.tensor_copy(vf[:, :], vp[:, :])
        nrow = min(P, HALF - blk * P)
        dma(val_half.ap()[blk * P:blk * P + nrow, :], vf[:nrow, :])

    # ============ pair AllGather -> full value table ============
    nc.gpsimd.collective_compute(
        "AllGather",
        mybir.AluOpType.bypass,
        replica_groups=[[0, 1], [2, 3], [4, 5], [6, 7]],
        ins=[val_half.ap()[:, :]],
        outs=[val_full.ap()[:, :]],
    )

    # ======================= P2: patch-table build ======================
    for h in range(NH):
        for l, (H_, W_) in enumerate(shapes):
            s = starts[l]
            for cy in (0, 1):
                for cx in (0, 1):
                    c = cy * 2 + cx
                    src = dap(val_full, (s + cy * W_ + cx) * EMB + h * HD,
                              [[W_ * EMB, H_ - 1], [EMB, W_ - 1], [1, HD]])
                    dst = dap(tableT, (h * L + s) * 4 * HD + c * HD,
                              [[W_ * 4 * HD, H_ - 1], [4 * HD, W_ - 1],
                               [1, HD]])
                    dma(dst, src)
            # fill never-gathered edge records (x=W-1 col, y=H-1 row) so the
            # table contains no uninitialized (possibly non-finite) bytes
            dma(dap(tableT, (h * L + s + W_ - 1) * 4 * HD,
                    [[W_ * 4 * HD, H_], [HD, 4], [1, HD]]),
                dap(val_full, (s + W_ - 1) * EMB + h * HD,
                    [[W_ * EMB, H_], [0, 4], [1, HD]]))
            dma(dap(tableT, (h * L + s + (H_ - 1) * W_) * 4 * HD,
                    [[4 * HD, W_ - 1], [HD, 4], [1, HD]]),
                dap(val_full, (s + (H_ - 1) * W_) * EMB + h * HD,
                    [[EMB, W_ - 1], [0, 4], [1, HD]]))

    # ==================== per-block frontend ====================
    def emit_frontend(blk):
        fq8 = apool.tile([P, EMB], i8, name="fq8", tag="fq8")
        dma(fq8, ins["feat_h"][blk * P:(blk + 1) * P, :])
        fq = apool.tile([P, EMB], f16, name="fq", tag="fq", bufs=3)
        nc.vector.tensor_scalar_mul(fq[:, :], fq8[:, :],
                                    rsct[:, blk, 0:1])
        # pos arrives as packed int4 nibble pairs (biased byte - 128, int8)
        HB = EMB // 2
        pq4 = apool.tile([P, HB], i8, name="pq4", tag="pq4")
        dma(pq4, ins["pos_h"][blk * P:(blk + 1) * P, :])
        p4f = apool.tile([P, HB], f32, name="p4f", tag="p4f")
        nc.vector.tensor_scalar_add(p4f[:, :], pq4[:, :], 128.0)
        tnib = apool.tile([P, HB], f32, name="tnib", tag="tnib")
        nc.vector.tensor_scalar_mul(tnib[:, :], p4f[:, :], 1.0 / 16.0)
        tni = apool.tile([P, HB], i32, name="tni", tag="tni")
        nc.vector.tensor_copy(tni[:, :], tnib[:, :])
        hi4 = apool.tile([P, HB], f32, name="hi4", tag="hi4")
        nc.vector.tensor_copy(hi4[:, :], tni[:, :])
        mfx = apool.tile([P, HB], f32, name="mfx", tag="mfx")
        nc.vector.tensor_tensor(mfx[:, :], tnib[:, :], hi4[:, :], op=op.is_lt)
        nc.vector.tensor_sub(hi4[:, :], hi4[:, :], mfx[:, :])
        lo4 = apool.tile([P, HB], f32, name="lo4", tag="lo4")
        nc.vector.scalar_tensor_tensor(lo4[:, :], hi4[:, :], -16.0, p4f[:, :],
                                       op0=op.mult, op1=op.add)
        pq = apool.tile([P, EMB], f16, name="pq", tag="pq")
        pstr = pq[:, :].ap[0][0]
        nc.vector.tensor_scalar(sap(pq[:, :], 0, [[pstr, P], [2, HB]]),
                                lo4[:, :], rsct[:, blk, 1:2],
                                m8sall[:, blk:blk + 1],
                                op0=op.mult, op1=op.add)
        nc.vector.tensor_scalar(sap(pq[:, :], 1, [[pstr, P], [2, HB]]),
                                hi4[:, :], rsct[:, blk, 1:2],
                                m8sall[:, blk:blk + 1],
                                op0=op.mult, op1=op.add)
        qb = apool.tile([P, EMB], f16, name="qb", tag="qb")
        nc.vector.tensor_add(qb[:, :], fq[:, :], pq[:, :])

        qtp = ps_tr.tile([P, 2, P], f16, name="qtp", tag="tr")
        nc.tensor.transpose(qtp[:, 0, :], qb[:, 0:P], idf16[:, :])
        nc.tensor.transpose(qtp[:, 1, :], qb[:, P:EMB], idf16[:, :])
        qts = apool.tile([P, 2, P], f16, name="qts", tag="qts", bufs=2)
        nc.vector.tensor_copy(qts[:, :, :], qtp[:, :, :])

        offp = ps_mm.tile([P, EMB], f32, name="offp", tag="mm")
        mm(offp, [(qts[:, 0, :], Woff[:, 0, :]), (qts[:, 1, :], Woff[:, 1, :])],
           bias=boff[:1, :])
        off = wpool.tile([P, EMB], f32, name="off", tag="off")
        nc.vector.tensor_copy(off[:, :], offp[:, :])

        attp = ps_sm.tile([P, NH * 16], f32, name="attp", tag="sm")
        mm(attp, [(qts[:, 0, :], Watt[:, 0, :]), (qts[:, 1, :], Watt[:, 1, :])],
           bias=batt[:1, :])
        att = wpool.tile([P, NH, 16], f32, name="att", tag="att")
        nc.vector.tensor_copy(att[:, :, :], attp[:, :].rearrange(
            "p (h l) -> p h l", h=NH))

        # softmax over (l,pt) per head
        rmax = wpool.tile([P, NH], f32, name="rmax", tag="rmax")
        nc.vector.reduce_max(rmax[:, :], att[:, :, :], axis=AX.X)
        exv = wpool.tile([P, NH, 16], f32, name="exv", tag="exv")
        rmaxa = rmax[:, :]
        nc.vector.tensor_sub(exv[:, :, :], att[:, :, :],
                             sap(rmaxa, 0, [rmaxa.ap[0], [1, NH], [0, 16]]))
        nc.scalar.activation(exv[:, :, :], exv[:, :, :], act_f.Exp)
        ssum = wpool.tile([P, NH], f32, name="ssum", tag="ssum")
        nc.vector.reduce_sum(ssum[:, :], exv[:, :, :], axis=AX.X)
        rsum = wpool.tile([P, NH], f32, name="rsum", tag="rsum")
        nc.vector.reciprocal(rsum[:, :], ssum[:, :])
        aw = wpool.tile([P, NH, 16], f32, name="aw", tag="aw")
        rsuma = rsum[:, :]
        nc.vector.tensor_mul(aw[:, :, :], exv[:, :, :],
                             sap(rsuma, 0, [rsuma.ap[0], [1, NH], [0, 16]]))

        def wt(name):
            return wpool.tile([P, EMB], f32, name=name, tag=name)

        loc = wt("loc")
        nc.vector.tensor_mul(loc[:, :], off[:, :], c_invn[:, :])
        refa = refr[:, blk, :]
        for xy in (0, 1):
            lvh = sap(loc[:, :], xy, [loc[:, :].ap[0], [32, NH], [8, NL],
                                      [2, NPT]])
            nc.vector.tensor_add(lvh, lvh,
                                 sap(refa, xy, [refa.ap[0], [0, NH], [2, NL],
                                                [0, NPT]]))
        pix = wt("pix")
        nc.vector.tensor_mul(pix[:, :], loc[:, :], c_pixs[:, :])
        nc.vector.tensor_scalar_add(pix[:, :], pix[:, :], -0.5)

        # floor(pix) robust to cast rounding mode
        xi = wpool.tile([P, EMB], i32, name="xi", tag="xi")
        nc.vector.tensor_copy(xi[:, :], pix[:, :])
        base = wt("base")
        nc.vector.tensor_copy(base[:, :], xi[:, :])
        fixm = wt("fixm")
        nc.vector.tensor_tensor(fixm[:, :], pix[:, :], base[:, :], op=op.is_lt)
        nc.vector.tensor_sub(base[:, :], base[:, :], fixm[:, :])
        wfrac = wt("wfrac")
        nc.vector.tensor_sub(wfrac[:, :], pix[:, :], base[:, :])

        basec = wt("basec")
        nc.vector.tensor_scalar_max(basec[:, :], base[:, :], 0.0)
        nc.vector.tensor_tensor(basec[:, :], basec[:, :], c_clip[:, :],
                                op=op.min)

        v0b = wt("v0b")
        nc.vector.tensor_tensor(v0b[:, :], base[:, :], c_vmax[:, :],
                                op=op.is_le)
        vld0 = wt("vld0")
        nc.vector.scalar_tensor_tensor(vld0[:, :], base[:, :], 0.0, v0b[:, :],
                                       op0=op.is_ge, op1=op.mult)
        v1b = wt("v1b")
        nc.vector.tensor_tensor(v1b[:, :], base[:, :], c_clip[:, :],
                                op=op.is_le)
        vld1 = wt("vld1")
        nc.vector.scalar_tensor_tensor(vld1[:, :], base[:, :], -1.0, v1b[:, :],
                                       op0=op.is_ge, op1=op.mult)

        tsh = wt("tsh")
        nc.vector.tensor_sub(tsh[:, :], base[:, :], basec[:, :])
        e0 = wt("e0")
        nc.vector.tensor_scalar(e0[:, :], tsh[:, :], 0.0, None,
                                op0=op.is_equal)
        em1 = wt("em1")
        nc.vector.tensor_scalar(em1[:, :], tsh[:, :], -1.0, None,
                                op0=op.is_equal)
        ep1 = wt("ep1")
        nc.vector.tensor_scalar(ep1[:, :], tsh[:, :], 1.0, None,
                                op0=op.is_equal)

        u0 = wt("u0")
        nc.vector.tensor_scalar(u0[:, :], wfrac[:, :], -1.0, 1.0, op0=op.mult,
                                op1=op.add)
        nc.vector.tensor_mul(u0[:, :], u0[:, :], vld0[:, :])
        u1 = wt("u1")
        nc.vector.tensor_mul(u1[:, :], wfrac[:, :], vld1[:, :])

        a0 = wt("a0")
        nc.vector.tensor_mul(a0[:, :], u0[:, :], e0[:, :])
        t1 = wt("t1")
        nc.vector.tensor_mul(t1[:, :], u1[:, :], em1[:, :])
        nc.vector.tensor_add(a0[:, :], a0[:, :], t1[:, :])
        a1 = wt("a1")
        nc.vector.tensor_mul(a1[:, :], u0[:, :], ep1[:, :])
        nc.vector.tensor_mul(t1[:, :], u1[:, :], e0[:, :])
        nc.vector.tensor_add(a1[:, :], a1[:, :], t1[:, :])

        def ycols(t):
            return sap(t[:, :], 1, [[t[:, :].ap[0][0], P], [2, P]])

        def xcols(t):
            return sap(t[:, :], 0, [[t[:, :].ap[0][0], P], [2, P]])

        awf = aw.rearrange("p h l -> p (h l)")
        ay0 = wpool.tile([P, P], f32, name="ay0", tag="ay0")
        nc.vector.tensor_mul(ay0[:, :], ycols(a0), awf)
        ay1 = wpool.tile([P, P], f32, name="ay1", tag="ay1")
        nc.vector.tensor_mul(ay1[:, :], ycols(a1), awf)

        w4 = wpool.tile([P, P, 4], f16, name="w4", tag="w4", bufs=2)
        nc.vector.tensor_mul(w4[:, :, 0], ay0[:, :], xcols(a0))
        nc.vector.tensor_mul(w4[:, :, 1], ay0[:, :], xcols(a1))
        nc.vector.tensor_mul(w4[:, :, 2], ay1[:, :], xcols(a0))
        nc.vector.tensor_mul(w4[:, :, 3], ay1[:, :], xcols(a1))

        cell = wpool.tile([P, P], f32, name="cell", tag="cell")
        nc.vector.tensor_mul(cell[:, :], ycols(basec), c_W[:, :])
        nc.vector.tensor_add(cell[:, :], cell[:, :], xcols(basec))
        nc.vector.tensor_add(cell[:, :], cell[:, :], c_S[:, :])

        nc.vector.tensor_add(cell[:, :], cell[:, :], c_HL[:, :])
        offs = wpool.tile([P, P], i32, name="offs", tag="offs", bufs=2)
        nc.vector.tensor_copy(offs[:, :], cell[:, :])
        return fq, w4, offs

    # ==================== LayerNorm ====================
    def emit_ln(r, gt, bt, pfx):
        nsum = opool.tile([P, 1], f32, name=f"{pfx}ns", tag=f"{pfx}ns")
        nc.vector.tensor_reduce(nsum[:, :], r[:, :], axis=AX.X, op=op.add,
                                negate=True)
        nmean = opool.tile([P, 1], f32, name=f"{pfx}nm", tag=f"{pfx}nm")
        nc.scalar.mul(nmean[:, :], nsum[:, :], 1.0 / EMB)
        c = opool.tile([P, EMB], f32, name=f"{pfx}c", tag=f"{pfx}c")
        nc.vector.tensor_scalar_add(c[:, :], r[:, :], nmean[:, :])
        csq = opool.tile([P, EMB], f32, name=f"{pfx}sq", tag=f"{pfx}sq")
        ssq = opool.tile([P, 1], f32, name=f"{pfx}ssq", tag=f"{pfx}ssq")
        nc.scalar.activation(csq[:, :], c[:, :], act_f.Square,
                             accum_out=ssq[:, :])
        std = opool.tile([P, 1], f32, name=f"{pfx}std", tag=f"{pfx}std")
        nc.scalar.activation(std[:, :], ssq[:, :], act_f.Sqrt,
                             bias=eps_t[:, :], scale=1.0 / EMB)
        rstd = opool.tile([P, 1], f32, name=f"{pfx}rs", tag=f"{pfx}rs")
        nc.vector.reciprocal(rstd[:, :], std[:, :])
        x = opool.tile([P, EMB], f32, name=f"{pfx}x", tag=f"{pfx}x")
        nc.vector.scalar_tensor_tensor(x[:, :], c[:, :], rstd[:, :], gt[:, :],
                                       op0=op.mult, op1=op.mult)
        nc.vector.tensor_add(x[:, :], x[:, :], bt[:, :])
        return x

    # ==================== per-group pipeline ====================
    def emit_group(g):
        blk = g
        fq, w4, offs = emit_frontend(blk)
        gb = gpool.tile([P, P, 4 * HD], f16, name="gb", tag="gb", bufs=2)
        for s in range(P):
            nc.gpsimd.indirect_dma_start(
                out=gb[:, s, :], out_offset=None,
                in_=tableT.ap()[:, :],
                in_offset=bass.IndirectOffsetOnAxis(ap=offs[:, s:s + 1],
                                                    axis=0))

        acat = kpool.tile([P, EMB], f32, name="acat", tag="acat")
        # all-heads combine, reduction tree folded in place inside gb
        gba = gb[:, :, :]
        pstr = gba.ap[0][0]

        def gsl(off, dims):
            return sap(gba, off, [[pstr, P]] + dims)

        # weights: w4 [P, (h,lp), 4] broadcast over head_dim (0-stride)
        w4b = sap(w4[:, :, :], 0,
                  [[w4[:, :, :].ap[0][0], P], [4, P], [1, 4], [0, HD]])
        gall = gsl(0, [[128, P], [HD, 4], [1, HD]])
        nc.vector.tensor_mul(gall, gall, w4b)
        # corner folds: c0+=c1, c2+=c3, c0+=c2
        d2 = [[128, P], [1, HD]]
        nc.vector.tensor_add(gsl(0, d2), gsl(0, d2), gsl(HD, d2))
        nc.vector.tensor_add(gsl(2 * HD, d2), gsl(2 * HD, d2), gsl(3 * HD, d2))
        nc.vector.tensor_add(gsl(0, d2), gsl(0, d2), gsl(2 * HD, d2))
        # lp folds: 16 -> 8 -> 4 -> 2 (per head; h stride 16*128)
        for w in (8, 4, 2):
            dh = [[16 * 128, NH], [128, w], [1, HD]]
            nc.vector.tensor_add(gsl(0, dh), gsl(0, dh), gsl(w * 128, dh))
        # final fold writes the fp32 attention output slice layout
        acv = sap(acat[:, :], 0, [[acat[:, :].ap[0][0], P], [HD, NH], [1, HD]])
        dh1 = [[16 * 128, NH], [1, HD]]
        nc.vector.tensor_add(acv, gsl(0, dh1), gsl(128, dh1))

        # ---- output projection + LN + FFN + LN ----
        ac16 = opool.tile([P, EMB], f16, name="ac16", tag="ac16")
        nc.vector.tensor_copy(ac16[:, :], acat[:, :])
        atp = ps_tr.tile([P, 2, P], f16, name="atp", tag="tr")
        nc.tensor.transpose(atp[:, 0, :], ac16[:, 0:P], idf16[:, :])
        nc.tensor.transpose(atp[:, 1, :], ac16[:, P:EMB], idf16[:, :])
        ats = opool.tile([P, 2, P], f16, name="ats", tag="ats")
        nc.vector.tensor_copy(ats[:, :, :], atp[:, :, :])
        oprj = ps_mm.tile([P, EMB], f32, name="oprj", tag="mm")
        mm(oprj, [(ats[:, 0, :], Wout[:, 0, :]),
                  (ats[:, 1, :], Wout[:, 1, :])], bias=bout[:1, :])

        r1 = opool.tile([P, EMB], f32, name="r1", tag="r1")
        nc.vector.tensor_add(r1[:, :], oprj[:, :], fq[:, :])
        x1 = emit_ln(r1, ln1g, ln1b, "la")

        x16 = opool.tile([P, EMB], f16, name="x16", tag="x16")
        nc.vector.tensor_copy(x16[:, :], x1[:, :])
        xtp = ps_tr.tile([P, 2, P], f16, name="xtp", tag="tr")
        nc.tensor.transpose(xtp[:, 0, :], x16[:, 0:P], idf16[:, :])
        nc.tensor.transpose(xtp[:, 1, :], x16[:, P:EMB], idf16[:, :])
        xts = opool.tile([P, 2, P], f16, name="xts", tag="xts")
        nc.vector.tensor_copy(xts[:, :, :], xtp[:, :, :])

        h1s = opool.tile([P, DFFN // P, P], f16, name="h1s", tag="h1s")
        hp = ps_mm.tile([P, DFFN // P, P], f32, name="hp", tag="hpw", bufs=1)
        for mt in range(DFFN // P):
            nc.tensor.matmul(hp[:, mt, :], W1[:, 0, mt * P:(mt + 1) * P],
                             xts[:, 0, :], start=True, stop=False)
            nc.tensor.matmul(hp[:, mt, :], W1[:, 1, mt * P:(mt + 1) * P],
                             xts[:, 1, :], start=False, stop=False)
            nc.tensor.matmul(hp[:, mt, :], b1r[:1, mt * P:(mt + 1) * P],
                             onesr[:1, :], start=False, stop=True)
        nc.scalar.activation(h1s[:, :, :], hp[:, :, :], act_f.Relu)

        yp = ps_mm.tile([P, EMB], f32, name="yp", tag="mm")
        for mt in range(DFFN // P):
            nc.tensor.matmul(yp[:, :], h1s[:, mt, :], W2[:, mt, :],
                             start=(mt == 0), stop=False)
        nc.tensor.matmul(yp[:, :], onesr[:1, :], b2r[:1, :],
                         start=False, stop=True)

        r2 = opool.tile([P, EMB], f32, name="r2", tag="r2")
        nc.vector.tensor_add(r2[:, :], yp[:, :], x1[:, :])
        x2 = emit_ln(r2, ln2g, ln2b, "lb")

        # per-row int8 quantization: scale = rowmax/127, shipped alongside
        absx = opool.tile([P, EMB], f32, name="absx", tag="absx")
        nc.scalar.activation(absx[:, :], x2[:, :], act_f.Abs)
        rmax = opool.tile([P, 1], f32, name="rmax2", tag="rmax2")
        nc.vector.reduce_max(rmax[:, :], absx[:, :], axis=AX.X)
        nc.vector.tensor_scalar_max(rmax[:, :], rmax[:, :], 1e-6)
        rinv = opool.tile([P, 1], f32, name="rinv", tag="rinv")
        nc.vector.reciprocal(rinv[:, :], rmax[:, :])
        smul = opool.tile([P, 1], f32, name="smul", tag="smul")
        nc.scalar.mul(smul[:, :], rinv[:, :], 127.0)
        osc = opool.tile([P, 1], f16, name="osc", tag="osc")
        nc.scalar.mul(osc[:, :], rmax[:, :], 1.0 / 127.0)
        # round(x*smul) = floor(x*smul + 0.5), floor robust to cast mode
        tq = opool.tile([P, EMB], f32, name="tq", tag="tq")
        nc.vector.tensor_scalar(tq[:, :], x2[:, :], smul[:, :], 0.5,
                                op0=op.mult, op1=op.add)
        qi = opool.tile([P, EMB], i32, name="qi", tag="qi")
        nc.vector.tensor_copy(qi[:, :], tq[:, :])
        qf = opool.tile([P, EMB], f32, name="qf", tag="qf")
        nc.vector.tensor_copy(qf[:, :], qi[:, :])
        qm = opool.tile([P, EMB], f32, name="qm", tag="qm")
        nc.vector.tensor_tensor(qm[:, :], tq[:, :], qf[:, :], op=op.is_lt)
        nc.vector.tensor_sub(qf[:, :], qf[:, :], qm[:, :])
        x2q = opool.tile([P, EMB], i8, name="x2q", tag="x2q")
        nc.vector.tensor_copy(x2q[:, :], qf[:, :])
        dma(outs["out_q"][blk * P:(blk + 1) * P, :], x2q)
        dma(outs["out_s"][blk * P:(blk + 1) * P, :], osc)

    for g in range(NGRP):
        emit_group(g)

    ctx.close()


# ------------------------------------------------------------ host entry ---

_CACHE = {}


def build_nc(cfg):
    import concourse.bass as bass
    from concourse import bacc, mybir, tile

    nc = bacc.Bacc("TRN2", debug=False, num_devices=8)
    f32 = mybir.dt.float32
    f16 = mybir.dt.float16

    def di(name, shape, dt=None):
        return nc.dram_tensor(name, list(shape), dt or f32,
                              kind="ExternalInput").ap()

    i8 = mybir.dt.int8
    HQ = cfg["HQ"]
    ins = dict(
        feat_h=di("feat_h", [HQ, EMB], i8),
        pos_h=di("pos_h", [HQ, EMB // 2], i8),
        rsc=di("rsc", [HQ, 2]),
        ref_q=di("ref_q", [HQ, NL, 2], f16),
        wchunk=di("wchunk", [WCHUNK], f16),
        sb16=di("sb16", [1, SB16TOT], f16),
        sb32=di("sb32", [1, SB32TOT]),
    )
    outs = dict(
        out_q=nc.dram_tensor("out_q", [HQ, EMB], i8,
                             kind="ExternalOutput").ap(),
        out_s=nc.dram_tensor("out_s", [HQ, 1], f16,
                             kind="ExternalOutput").ap(),
    )
    with tile.TileContext(nc) as tc:
        emit_kernel(tc, outs, ins, cfg)
    nc.compile()
    return nc


def make_in_maps(inputs, cfg):
    feats = np.asarray(inputs["features"], np.float32)
    pos = np.asarray(inputs["pos"], np.float32)
    refp = np.asarray(inputs["reference_points"], np.float32)
    B = feats.shape[0]
    HQ, L = cfg["HQ"], cfg["L"]
    half = L // 2

    consts = host_constants(cfg)
    wblob = np.concatenate(
        [np.asarray(inputs[k], np.float32).astype(np.float16).reshape(-1)
         for k in WORDER])
    assert wblob.size == WTOT

    sb16src = dict(b_val=inputs["b_val"], b_off=inputs["b_off"],
                   b_attn=inputs["b_attn"], b_out=inputs["b_out"],
                   b1=inputs["b1"], b2=inputs["b2"],
                   ones_row=consts["ones_row"], ident=consts["ident"])
    sb16 = np.concatenate(
        [np.asarray(sb16src[n], np.float32).reshape(-1)
         for n, _ in SB16ORD]).astype(np.float16).reshape(1, -1)
    assert sb16.size == SB16TOT
    sb32src = dict(ln1_g=inputs["ln1_g"], ln1_b=inputs["ln1_b"],
                   ln2_g=inputs["ln2_g"], ln2_b=inputs["ln2_b"],
                   cst_xy=consts["cst_xy"], cst_hlp=consts["cst_hlp"])
    sb32 = np.concatenate(
        [np.asarray(sb32src[n], np.float32).reshape(-1)
         for n, _ in SB32ORD]).astype(np.float32).reshape(1, -1)
    assert sb32.size == SB32TOT

    def quant_rows(x, nrow):
        # per-row symmetric int8: q = round(x/scale), scale = rowmax/127
        q = np.zeros((nrow, x.shape[1]), np.int8)
        sc = np.ones((nrow, 1), np.float32)
        mx = np.abs(x).max(axis=1, keepdims=True)
        mx = np.maximum(mx, 1e-12)
        sc[:x.shape[0]] = (mx / 127.0).astype(np.float32)
        q[:x.shape[0]] = np.clip(np.rint(x / (mx / 127.0)), -127, 127
                                 ).astype(np.int8)
        return q, sc

    def quant_pos4(x, nrow):
        # per-row int4 nibbles: v = clip(round(x/s), -7, 7)+8, s = rowmax/7;
        # packed pairs (lo | hi<<4) shipped as biased int8 (byte - 128)
        pk = np.zeros((nrow, x.shape[1] // 2), np.int8)
        sc = np.ones((nrow, 1), np.float32)
        mx = np.maximum(np.abs(x).max(axis=1, keepdims=True), 1e-12)
        s = (mx / 7.0).astype(np.float32)
        q = (np.clip(np.rint(x / s), -7, 7) + 8).astype(np.int16)
        by = q[:, 0::2] + (q[:, 1::2] << 4)
        pk[:x.shape[0]] = (by - 128).astype(np.int8)
        pk[x.shape[0]:] = 8 + (8 << 4) - 128
        sc[:x.shape[0]] = s
        return pk, sc

    halves = [(0, half), (half, L)]
    in_maps = []
    for core in range(2 * B):
        b, hf = core // 2, core % 2
        s, e = halves[hf]
        fh, fsc = quant_rows(feats[b, s:e], HQ)
        ph, psc = quant_pos4(pos[b, s:e], HQ)
        rq = np.zeros((HQ, NL, 2), np.float16)
        rq[:e - s] = refp[b, s:e].astype(np.float16)
        m = dict(feat_h=fh, pos_h=ph, ref_q=rq,
                 rsc=np.ascontiguousarray(np.hstack([fsc, psc])),
                 wchunk=np.ascontiguousarray(
                     wblob[core * WCHUNK:(core + 1) * WCHUNK]),
                 sb16=sb16, sb32=sb32)
        in_maps.append(m)
    return in_maps, halves


def assemble_out(res, B, L, halves):
    out = np.zeros((B, L, EMB), np.float32)
    for core in range(2 * B):
        b, hf = core // 2, core % 2
        s, e = halves[hf]
        n = e - s
        q = res.results[core]["out_q"][:n].astype(np.float32)
        sc = res.results[core]["out_s"][:n]
        out[b, s:e] = q * sc
    return out


def kernel(**inputs):
    from concourse import bass_utils

    cfg = CFG_FULL
    in_maps, halves = make_in_maps(inputs, cfg)
    B = np.asarray(inputs["features"]).shape[0]
    L = cfg["L"]

    if "nc" not in _CACHE:
        _CACHE["nc"] = build_nc(cfg)
    nc = _CACHE["nc"]

    res = bass_utils.run_bass_kernel_spmd(nc, in_maps,
                                          core_ids=list(range(2 * B)))
    return assemble_out(res, B, L, halves)


# revision 38
# speedup vs baseline: 1.2530x; 1.2530x over previous
"""Trainium2 Bass kernel for a Deformable-DETR style encoder block.

Sharding: 8 NeuronCores = 4 batch samples x 2 query-halves.

The dispatch is wire-transfer-bound (axon tunnel), so inputs/outputs are
aggressively compressed and nothing is uploaded twice:
  - features: per-row-scaled int8, own half only; the value projection is
    computed per half and the full per-batch table assembled on-device via
    a pair AllGather (cores 2b <-> 2b+1).
  - pos: per-row-scaled int4 nibble pairs (unpacked arithmetically on DVE);
    reference points fp16.
  - weights: fp16, uploaded sharded 1/8 per core and reassembled with an
    8-way AllGather; all matmuls run fp16 (PSUM accumulates fp32).
  - output: per-row-scaled int8 + fp16 row scales, dequantized on host.

Per core:
  - value projection of own half -> pair AllGather -> fp16 "patch table"
    in DRAM: for cell (y,x) and head h the 2x2 neighborhood [V[y,x],
    V[y,x+1], V[y+1,x], V[y+1,x+1]] is packed contiguously (4*32 fp16 =
    256B), so one dma_gather descriptor fetches a complete bilinear patch.
  - offset/attention projections, softmax, bilinear weights and cell
    indices computed query-major (PE transposes feed the matmuls).
  - gpsimd indirect DMA fetches patches; DVE multiplies and tree-reduces.
  - output projection + LayerNorm + FFN + LayerNorm, int8 quant, DMA out.
"""

import numpy as np
from contextlib import ExitStack

EMB = 256
NH = 8
NL = 4
NPT = 4
HD = 32
DFFN = 1024
P = 128


def make_cfg(shapes, n_blk_q, grp):
    L = sum(h * w for h, w in shapes)
    starts = np.cumsum([0] + [h * w for h, w in shapes])[:-1].tolist()
    n_blk_full = -(-L // P)
    assert n_blk_q % grp == 0
    return dict(
        shapes=[tuple(s) for s in shapes], starts=starts, L=L,
        LPAD=n_blk_full * P, NBF=n_blk_full, NBQ=n_blk_q, HQ=n_blk_q * P,
        GRP=grp, NGRP=n_blk_q // grp,
    )


CFG_FULL = make_cfg([(100, 100), (50, 50), (25, 25), (13, 13)], 52, 1)
HALF = 6647

# weight blob layout: name -> (element offset, k // P, n), fp16 elements
WORDER = ["W_val", "W_off", "W_attn", "W_out", "W1", "W2"]
WSHAPES = {"W_val": (EMB, EMB), "W_off": (EMB, EMB),
           "W_attn": (EMB, NH * NL * NPT), "W_out": (EMB, EMB),
           "W1": (EMB, DFFN), "W2": (DFFN, EMB)}
WOFFS = {}
_off = 0
for _n in WORDER:
    _k, _c = WSHAPES[_n]
    WOFFS[_n] = (_off, _k // P, _c)
    _off += _k * _c
WTOT = _off
assert WTOT % 8 == 0
WCHUNK = WTOT // 8

# packed small-constant blobs (fp16 / fp32), offsets in elements
SB16ORD = [("b_val", EMB), ("b_off", EMB), ("b_attn", NH * NL * NPT),
           ("b_out", EMB), ("b1", DFFN), ("b2", EMB), ("ones_row", P),
           ("ident", P * P)]
SB16OFF = {}
_off = 0
for _n, _c in SB16ORD:
    SB16OFF[_n] = _off
    _off += _c
SB16TOT = _off
SB32ORD = [("ln1_g", EMB), ("ln1_b", EMB), ("ln2_g", EMB), ("ln2_b", EMB),
           ("cst_xy", 4 * EMB), ("cst_hlp", 3 * P)]
SB32OFF = {}
_off = 0
for _n, _c in SB32ORD:
    SB32OFF[_n] = _off
    _off += _c
SB32TOT = _off


# ------------------------------------------------------- host-side consts ---

def host_constants(cfg):
    shapes, starts = cfg["shapes"], cfg["starts"]
    invnorm = np.zeros(EMB, np.float32)
    pixscale = np.zeros(EMB, np.float32)
    clipmax = np.zeros(EMB, np.float32)
    vmax = np.zeros(EMB, np.float32)
    for h in range(NH):
        for l, (H_, W_) in enumerate(shapes):
            for pt in range(NPT):
                base = h * (NL * NPT * 2) + l * (NPT * 2) + pt * 2
                invnorm[base + 0] = 1.0 / W_
                invnorm[base + 1] = 1.0 / H_
                pixscale[base + 0] = W_
                pixscale[base + 1] = H_
                clipmax[base + 0] = W_ - 2
                clipmax[base + 1] = H_ - 2
                vmax[base + 0] = W_ - 1
                vmax[base + 1] = H_ - 1
    cst_xy = np.stack([invnorm, pixscale, clipmax, vmax])

    wrow = np.zeros(P, np.float32)
    srow = np.zeros(P, np.float32)
    hrow = np.zeros(P, np.float32)
    L = cfg["L"]
    for h in range(NH):
        for l, (H_, W_) in enumerate(shapes):
            for pt in range(NPT):
                base = h * (NL * NPT) + l * NPT + pt
                wrow[base] = W_
                srow[base] = starts[l]
                hrow[base] = h * L
    cst_hlp = np.stack([wrow, srow, hrow])

    ident = np.eye(P, dtype=np.float16)
    ones_row = np.ones((1, P), np.float16)
    return dict(cst_xy=cst_xy, cst_hlp=cst_hlp, ident=ident,
                ones_row=ones_row)


# ------------------------------------------------------------- emission ---

def emit_kernel(tc, outs, ins, cfg):
    import concourse.bass as bass
    from concourse import mybir

    nc = tc.nc
    op = mybir.AluOpType
    act_f = mybir.ActivationFunctionType
    f32, f16 = mybir.dt.float32, mybir.dt.float16
    i32 = mybir.dt.int32
    AX = mybir.AxisListType

    shapes, starts = cfg["shapes"], cfg["starts"]
    L, NBQ, NGRP = (cfg[k] for k in ("L", "NBQ", "NGRP"))

    ctx = ExitStack()

    def dap(handle, offset, dims):
        return bass.AP(tensor=handle, offset=offset,
                       ap=[list(d) for d in dims])

    def sap(ap0, extra_off, dims):
        return bass.AP(tensor=ap0.tensor, offset=ap0.offset + extra_off,
                       ap=[list(d) for d in dims])

    i8 = mybir.dt.int8

    # ---- internal DRAM ----
    val_half = nc.dram_tensor("val_half", [HALF, EMB], f16, kind="Internal")
    val_full = nc.dram_tensor("val_full", [2 * HALF, EMB], f16,
                              kind="Internal")
    tableT = nc.dram_tensor("tableT", [NH * L, 4 * HD], f16, kind="Internal")
    wb_in = nc.dram_tensor("wb_in", [WCHUNK], f16, kind="Internal")
    wblob = nc.dram_tensor("wblob", [8 * WCHUNK], f16, kind="Internal")

    # ---- pools ----
    cpool = ctx.enter_context(tc.tile_pool(name="consts", bufs=1))
    apool = ctx.enter_context(tc.tile_pool(name="acts", bufs=3))
    wpool = ctx.enter_context(tc.tile_pool(name="wmath", bufs=1))
    gpool = ctx.enter_context(tc.tile_pool(name="gath", bufs=2))
    kpool = ctx.enter_context(tc.tile_pool(name="comb", bufs=2))
    opool = ctx.enter_context(tc.tile_pool(name="outp", bufs=2))
    ps_tr = ctx.enter_context(tc.tile_pool(name="ps_tr", bufs=2, space="PSUM"))
    ps_mm = ctx.enter_context(tc.tile_pool(name="ps_mm", bufs=2, space="PSUM"))
    ps_sm = ctx.enter_context(tc.tile_pool(name="ps_sm", bufs=2, space="PSUM"))

    def dma(out_ap, in_ap):
        nc.sync.dma_start(out=out_ap, in_=in_ap)

    # ---- weights: sharded upload, 8-core AllGather, then load from blob ----
    dma(wb_in.ap()[:], ins["wchunk"][:])
    nc.gpsimd.collective_compute(
        "AllGather",
        mybir.AluOpType.bypass,
        replica_groups=[[0, 1, 2, 3, 4, 5, 6, 7]],
        ins=[wb_in.ap()[:]],
        outs=[wblob.ap()[:]],
    )

    def load_w(name):
        base, a, n = WOFFS[name]
        t = cpool.tile([P, a, n], f16, name=f"s_{name}")
        dma(t, dap(wblob, base, [[n, P], [P * n, a], [1, n]]))
        return t

    Wval = load_w("W_val")
    Woff = load_w("W_off")
    Watt = load_w("W_attn")
    Wout = load_w("W_out")
    W1 = load_w("W1")
    W2 = load_w("W2")

    sb16t = ins["sb16"].tensor
    sb32t = ins["sb32"].tensor

    def load_row(name, n):
        t = cpool.tile([1, n], f16, name=f"r_{name}")
        dma(t, dap(sb16t, SB16OFF[name], [[n, 1], [1, n]]))
        return t

    bval = load_row("b_val", EMB)
    boff = load_row("b_off", EMB)
    batt = load_row("b_attn", NH * NL * NPT)
    bout = load_row("b_out", EMB)
    b1r = load_row("b1", DFFN)
    b2r = load_row("b2", EMB)
    onesr = load_row("ones_row", P)

    def load_bc(off, n, name):
        t = cpool.tile([P, n], f32, name=f"b_{name}")
        dma(t, dap(sb32t, off, [[0, P], [1, n]]))
        return t

    ln1g = load_bc(SB32OFF["ln1_g"], EMB, "ln1g")
    ln1b = load_bc(SB32OFF["ln1_b"], EMB, "ln1b")
    ln2g = load_bc(SB32OFF["ln2_g"], EMB, "ln2g")
    ln2b = load_bc(SB32OFF["ln2_b"], EMB, "ln2b")
    c_invn = load_bc(SB32OFF["cst_xy"], EMB, "invn")
    c_pixs = load_bc(SB32OFF["cst_xy"] + EMB, EMB, "pixs")
    c_clip = load_bc(SB32OFF["cst_xy"] + 2 * EMB, EMB, "clip")
    c_vmax = load_bc(SB32OFF["cst_xy"] + 3 * EMB, EMB, "vmax")
    c_W = load_bc(SB32OFF["cst_hlp"], P, "cw")
    c_S = load_bc(SB32OFF["cst_hlp"] + P, P, "cs")
    c_HL = load_bc(SB32OFF["cst_hlp"] + 2 * P, P, "chl")

    idf16 = cpool.tile([P, P], f16, name="idf16")
    dma(idf16, dap(sb16t, SB16OFF["ident"], [[P, P], [1, P]]))
    eps_t = cpool.tile([P, 1], f32, name="eps_t")
    nc.vector.memset(eps_t[:, :], 1e-5)

    refr = cpool.tile([P, NBQ, 2 * NL], f16, name="refr")
    dma(refr, ins["ref_q"].rearrange("(b p) l c -> p b (l c)", p=P))
    rsct = cpool.tile([P, NBQ, 2], f32, name="rsct")
    dma(rsct, ins["rsc"].rearrange("(b p) c -> p b c", p=P))
    m8sall = cpool.tile([P, NBQ], f32, name="m8sall")
    nc.scalar.mul(m8sall[:, :], rsct[:, :, 1], -8.0)

    def mm(psum_ap, pairs, bias=None):
        seq = list(pairs)
        if bias is not None:
            seq.append((onesr[:1, :psum_ap.shape[0]], bias))
        for i, (lt, rt) in enumerate(seq):
            nc.tensor.matmul(psum_ap, lt, rt,
                             start=(i == 0), stop=(i == len(seq) - 1))

    # ============ P1: value projection of the own half ============
    for blk in range(NBQ):
        fi8 = apool.tile([P, EMB], i8, name="fi8", tag="fi8")
        dma(fi8, ins["feat_h"][blk * P:(blk + 1) * P, :])
        fv = apool.tile([P, EMB], f16, name="fv", tag="fv")
        nc.vector.tensor_scalar_mul(fv[:, :], fi8[:, :],
                                    rsct[:, blk, 0:1])
        ftp = ps_tr.tile([P, 2, P], f16, name="ftp", tag="tr")
        nc.tensor.transpose(ftp[:, 0, :], fv[:, 0:P], idf16[:, :])
        nc.tensor.transpose(ftp[:, 1, :], fv[:, P:EMB], idf16[:, :])
        fts = apool.tile([P, 2, P], f16, name="fts", tag="fts")
        nc.vector.tensor_copy(fts[:, :, :], ftp[:, :, :])
        vp = ps_mm.tile([P, EMB], f32, name="vp", tag="mm")
        mm(vp, [(fts[:, 0, :], Wval[:, 0, :]), (fts[:, 1, :], Wval[:, 1, :])],
           bias=bval[:1, :])
        vf = apool.tile([P, EMB], f16, name="vf", tag="vf")
        nc.vector.tensor_copy(vf[:, :], vp[:, :])
        nrow = min(P, HALF - blk * P)
        dma(val_half.ap()[blk * P:blk * P + nrow, :], vf[:nrow, :])

    # ============ pair AllGather -> full value table ============
    nc.gpsimd.collective_compute(
        "AllGather",
        mybir.AluOpType.bypass,
        replica_groups=[[0, 1], [2, 3], [4, 5], [6, 7]],
        ins=[val_half.ap()[:, :]],
        outs=[val_full.ap()[:, :]],
    )

    # ======================= P2: patch-table build ======================
    for h in range(NH):
        for l, (H_, W_) in enumerate(shapes):
            s = starts[l]
            for cy in (0, 1):
                for cx in (0, 1):
                    c = cy * 2 + cx
                    src = dap(val_full, (s + cy * W_ + cx) * EMB + h * HD,
                              [[W_ * EMB, H_ - 1], [EMB, W_ - 1], [1, HD]])
                    dst = dap(tableT, (h * L + s) * 4 * HD + c * HD,
                              [[W_ * 4 * HD, H_ - 1], [4 * HD, W_ - 1],
                               [1, HD]])
                    dma(dst, src)
            # fill never-gathered edge records (x=W-1 col, y=H-1 row) so the
            # table contains no uninitialized (possibly non-finite) bytes
            dma(dap(tableT, (h * L + s + W_ - 1) * 4 * HD,
                    [[W_ * 4 * HD, H_], [HD, 4], [1, HD]]),
                dap(val_full, (s + W_ - 1) * EMB + h * HD,
                    [[W_ * EMB, H_], [0, 4], [1, HD]]))
            dma(dap(tableT, (h * L + s + (H_ - 1) * W_) * 4 * HD,
                    [[4 * HD, W_ - 1], [HD, 4], [1, HD]]),
                dap(val_full, (s + (H_ - 1) * W_) * EMB + h * HD,
                    [[EMB, W_ - 1], [0, 4], [1, HD]]))

    # ==================== per-block frontend ====================
    def emit_frontend(blk):
        fq8 = apool.tile([P, EMB], i8, name="fq8", tag="fq8")
        dma(fq8, ins["feat_h"][blk * P:(blk + 1) * P, :])
        fq = apool.tile([P, EMB], f16, name="fq", tag="fq", bufs=3)
        nc.vector.tensor_scalar_mul(fq[:, :], fq8[:, :],
                                    rsct[:, blk, 0:1])
        # pos arrives as packed int4 nibble pairs (biased byte - 128, int8)
        HB = EMB // 2
        pq4 = apool.tile([P, HB], i8, name="pq4", tag="pq4")
        dma(pq4, ins["pos_h"][blk * P:(blk + 1) * P, :])
        p4f = apool.tile([P, HB], f32, name="p4f", tag="p4f")
        nc.vector.tensor_scalar_add(p4f[:, :], pq4[:, :], 128.0)
        tnib = apool.tile([P, HB], f32, name="tnib", tag="tnib")
        nc.vector.tensor_scalar_mul(tnib[:, :], p4f[:, :], 1.0 / 16.0)
        tni = apool.tile([P, HB], i32, name="tni", tag="tni")
        nc.vector.tensor_copy(tni[:, :], tnib[:, :])
        hi4 = apool.tile([P, HB], f32, name="hi4", tag="hi4")
        nc.vector.tensor_copy(hi4[:, :], tni[:, :])
        mfx = apool.tile([P, HB], f32, name="mfx", tag="mfx")
        nc.vector.tensor_tensor(mfx[:, :], tnib[:, :], hi4[:, :], op=op.is_lt)
        nc.vector.tensor_sub(hi4[:, :], hi4[:, :], mfx[:, :])
        lo4 = apool.tile([P, HB], f32, name="lo4", tag="lo4")
        nc.vector.scalar_tensor_tensor(lo4[:, :], hi4[:, :], -16.0, p4f[:, :],
                                       op0=op.mult, op1=op.add)
        pq = apool.tile([P, EMB], f16, name="pq", tag="pq")
        pstr = pq[:, :].ap[0][0]
        nc.vector.tensor_scalar(sap(pq[:, :], 0, [[pstr, P], [2, HB]]),
                                lo4[:, :], rsct[:, blk, 1:2],
                                m8sall[:, blk:blk + 1],
                                op0=op.mult, op1=op.add)
        nc.vector.tensor_scalar(sap(pq[:, :], 1, [[pstr, P], [2, HB]]),
                                hi4[:, :], rsct[:, blk, 1:2],
                                m8sall[:, blk:blk + 1],
                                op0=op.mult, op1=op.add)
        qb = apool.tile([P, EMB], f16, name="qb", tag="qb")
        nc.vector.tensor_add(qb[:, :], fq[:, :], pq[:, :])

        qtp = ps_tr.tile([P, 2, P], f16, name="qtp", tag="tr")
        nc.tensor.transpose(qtp[:, 0, :], qb[:, 0:P], idf16[:, :])
        nc.tensor.transpose(qtp[:, 1, :], qb[:, P:EMB], idf16[:, :])
        qts = apool.tile([P, 2, P], f16, name="qts", tag="qts", bufs=2)
        nc.vector.tensor_copy(qts[:, :, :], qtp[:, :, :])

        offp = ps_mm.tile([P, EMB], f32, name="offp", tag="mm")
        mm(offp, [(qts[:, 0, :], Woff[:, 0, :]), (qts[:, 1, :], Woff[:, 1, :])],
           bias=boff[:1, :])
        off = wpool.tile([P, EMB], f32, name="off", tag="off")
        nc.vector.tensor_copy(off[:, :], offp[:, :])

        attp = ps_sm.tile([P, NH * 16], f32, name="attp", tag="sm")
        mm(attp, [(qts[:, 0, :], Watt[:, 0, :]), (qts[:, 1, :], Watt[:, 1, :])],
           bias=batt[:1, :])
        att = wpool.tile([P, NH, 16], f32, name="att", tag="att")
        nc.vector.tensor_copy(att[:, :, :], attp[:, :].rearrange(
            "p (h l) -> p h l", h=NH))

        # softmax over (l,pt) per head
        rmax = wpool.tile([P, NH], f32, name="rmax", tag="rmax")
        nc.vector.reduce_max(rmax[:, :], att[:, :, :], axis=AX.X)
        exv = wpool.tile([P, NH, 16], f32, name="exv", tag="exv")
        rmaxa = rmax[:, :]
        nc.vector.tensor_sub(exv[:, :, :], att[:, :, :],
                             sap(rmaxa, 0, [rmaxa.ap[0], [1, NH], [0, 16]]))
        nc.scalar.activation(exv[:, :, :], exv[:, :, :], act_f.Exp)
        ssum = wpool.tile([P, NH], f32, name="ssum", tag="ssum")
        nc.vector.reduce_sum(ssum[:, :], exv[:, :, :], axis=AX.X)
        rsum = wpool.tile([P, NH], f32, name="rsum", tag="rsum")
        nc.vector.reciprocal(rsum[:, :], ssum[:, :])
        aw = wpool.tile([P, NH, 16], f32, name="aw", tag="aw")
        rsuma = rsum[:, :]
        nc.vector.tensor_mul(aw[:, :, :], exv[:, :, :],
                             sap(rsuma, 0, [rsuma.ap[0], [1, NH], [0, 16]]))

        def wt(name):
            return wpool.tile([P, EMB], f32, name=name, tag=name)

        loc = wt("loc")
        nc.vector.tensor_mul(loc[:, :], off[:, :], c_invn[:, :])
        refa = refr[:, blk, :]
        for xy in (0, 1):
            lvh = sap(loc[:, :], xy, [loc[:, :].ap[0], [32, NH], [8, NL],
                                      [2, NPT]])
            nc.vector.tensor_add(lvh, lvh,
                                 sap(refa, xy, [refa.ap[0], [0, NH], [2, NL],
                                                [0, NPT]]))
        pix = wt("pix")
        nc.vector.tensor_mul(pix[:, :], loc[:, :], c_pixs[:, :])
        nc.vector.tensor_scalar_add(pix[:, :], pix[:, :], -0.5)

        # floor(pix) robust to cast rounding mode
        xi = wpool.tile([P, EMB], i32, name="xi", tag="xi")
        nc.vector.tensor_copy(xi[:, :], pix[:, :])
        base = wt("base")
        nc.vector.tensor_copy(base[:, :], xi[:, :])
        fixm = wt("fixm")
        nc.vector.tensor_tensor(fixm[:, :], pix[:, :], base[:, :], op=op.is_lt)
        nc.vector.tensor_sub(base[:, :], base[:, :], fixm[:, :])
        wfrac = wt("wfrac")
        nc.vector.tensor_sub(wfrac[:, :], pix[:, :], base[:, :])

        basec = wt("basec")
        nc.vector.tensor_scalar_max(basec[:, :], base[:, :], 0.0)
        nc.vector.tensor_tensor(basec[:, :], basec[:, :], c_clip[:, :],
                                op=op.min)

        v0b = wt("v0b")
        nc.vector.tensor_tensor(v0b[:, :], base[:, :], c_vmax[:, :],
                                op=op.is_le)
        vld0 = wt("vld0")
        nc.vector.scalar_tensor_tensor(vld0[:, :], base[:, :], 0.0, v0b[:, :],
                                       op0=op.is_ge, op1=op.mult)
        v1b = wt("v1b")
        nc.vector.tensor_tensor(v1b[:, :], base[:, :], c_clip[:, :],
                                op=op.is_le)
        vld1 = wt("vld1")
        nc.vector.scalar_tensor_tensor(vld1[:, :], base[:, :], -1.0, v1b[:, :],
                                       op0=op.is_ge, op1=op.mult)

        tsh = wt("tsh")
        nc.vector.tensor_sub(tsh[:, :], base[:, :], basec[:, :])
        e0 = wt("e0")
        nc.vector.tensor_scalar(e0[:, :], tsh[:, :], 0.0, None,
                                op0=op.is_equal)
        em1 = wt("em1")
        nc.vector.tensor_scalar(em1[:, :], tsh[:, :], -1.0, None,
                                op0=op.is_equal)
        ep1 = wt("ep1")
        nc.vector.tensor_scalar(ep1[:, :], tsh[:, :], 1.0, None,
                                op0=op.is_equal)

        u0 = wt("u0")
        nc.vector.tensor_scalar(u0[:, :], wfrac[:, :], -1.0, 1.0, op0=op.mult,
                                op1=op.add)
        nc.vector.tensor_mul(u0[:, :], u0[:, :], vld0[:, :])
        u1 = wt("u1")
        nc.vector.tensor_mul(u1[:, :], wfrac[:, :], vld1[:, :])

        a0 = wt("a0")
        nc.vector.tensor_mul(a0[:, :], u0[:, :], e0[:, :])
        t1 = wt("t1")
        nc.vector.tensor_mul(t1[:, :], u1[:, :], em1[:, :])
        nc.vector.tensor_add(a0[:, :], a0[:, :], t1[:, :])
        a1 = wt("a1")
        nc.vector.tensor_mul(a1[:, :], u0[:, :], ep1[:, :])
        nc.vector.tensor_mul(t1[:, :], u1[:, :], e0[:, :])
        nc.vector.tensor_add(a1[:, :], a1[:, :], t1[:, :])

        def ycols(t):
            return sap(t[:, :], 1, [[t[:, :].ap[0][0], P], [2, P]])

        def xcols(t):
            return sap(t[:, :], 0, [[t[:, :].ap[0][0], P], [2, P]])

        awf = aw.rearrange("p h l -> p (h l)")
        ay0 = wpool.tile([P, P], f32, name="ay0", tag="ay0")
        nc.vector.tensor_mul(ay0[:, :], ycols(a0), awf)
        ay1 = wpool.tile([P, P], f32, name="ay1", tag="ay1")
        nc.vector.tensor_mul(ay1[:, :], ycols(a1), awf)

        w4 = wpool.tile([P, P, 4], f16, name="w4", tag="w4", bufs=2)
        nc.vector.tensor_mul(w4[:, :, 0], ay0[:, :], xcols(a0))
        nc.vector.tensor_mul(w4[:, :, 1], ay0[:, :], xcols(a1))
        nc.vector.tensor_mul(w4[:, :, 2], ay1[:, :], xcols(a0))
        nc.vector.tensor_mul(w4[:, :, 3], ay1[:, :], xcols(a1))

        cell = wpool.tile([P, P], f32, name="cell", tag="cell")
        nc.vector.tensor_mul(cell[:, :], ycols(basec), c_W[:, :])
        nc.vector.tensor_add(cell[:, :], cell[:, :], xcols(basec))
        nc.vector.tensor_add(cell[:, :], cell[:, :], c_S[:, :])

        nc.vector.tensor_add(cell[:, :], cell[:, :], c_HL[:, :])
        offs = wpool.tile([P, P], i32, name="offs", tag="offs", bufs=2)
        nc.vector.tensor_copy(offs[:, :], cell[:, :])
        return fq, w4, offs

    # ==================== LayerNorm ====================
    def emit_ln(r, gt, bt, pfx):
        nsum = opool.tile([P, 1], f32, name=f"{pfx}ns", tag=f"{pfx}ns")
        nc.vector.tensor_reduce(nsum[:, :], r[:, :], axis=AX.X, op=op.add,
                                negate=True)
        nmean = opool.tile([P, 1], f32, name=f"{pfx}nm", tag=f"{pfx}nm")
        nc.scalar.mul(nmean[:, :], nsum[:, :], 1.0 / EMB)
        c = opool.tile([P, EMB], f32, name=f"{pfx}c", tag=f"{pfx}c")
        nc.vector.tensor_scalar_add(c[:, :], r[:, :], nmean[:, :])
        csq = opool.tile([P, EMB], f32, name=f"{pfx}sq", tag=f"{pfx}sq")
        ssq = opool.tile([P, 1], f32, name=f"{pfx}ssq", tag=f"{pfx}ssq")
        nc.scalar.activation(csq[:, :], c[:, :], act_f.Square,
                             accum_out=ssq[:, :])
        std = opool.tile([P, 1], f32, name=f"{pfx}std", tag=f"{pfx}std")
        nc.scalar.activation(std[:, :], ssq[:, :], act_f.Sqrt,
                             bias=eps_t[:, :], scale=1.0 / EMB)
        rstd = opool.tile([P, 1], f32, name=f"{pfx}rs", tag=f"{pfx}rs")
        nc.vector.reciprocal(rstd[:, :], std[:, :])
        x = opool.tile([P, EMB], f32, name=f"{pfx}x", tag=f"{pfx}x")
        nc.vector.scalar_tensor_tensor(x[:, :], c[:, :], rstd[:, :], gt[:, :],
                                       op0=op.mult, op1=op.mult)
        nc.vector.tensor_add(x[:, :], x[:, :], bt[:, :])
        return x

    # ==================== per-group pipeline ====================
    def emit_group(g):
        blk = g
        fq, w4, offs = emit_frontend(blk)
        gb = gpool.tile([P, P, 4 * HD], f16, name="gb", tag="gb", bufs=2)
        for s in range(P):
            nc.gpsimd.indirect_dma_start(
                out=gb[:, s, :], out_offset=None,
                in_=tableT.ap()[:, :],
                in_offset=bass.IndirectOffsetOnAxis(ap=offs[:, s:s + 1],
                                                    axis=0))

        acat = kpool.tile([P, EMB], f32, name="acat", tag="acat")
        # all-heads combine, reduction tree folded in place inside gb
        gba = gb[:, :, :]
        pstr = gba.ap[0][0]

        def gsl(off, dims):
            return sap(gba, off, [[pstr, P]] + dims)

        # weights: w4 [P, (h,lp), 4] broadcast over head_dim (0-stride)
        w4b = sap(w4[:, :, :], 0,
                  [[w4[:, :, :].ap[0][0], P], [4, P], [1, 4], [0, HD]])
        gall = gsl(0, [[128, P], [HD, 4], [1, HD]])
        nc.vector.tensor_mul(gall, gall, w4b)
        # corner folds: c0+=c1, c2+=c3, c0+=c2
        d2 = [[128, P], [1, HD]]
        nc.vector.tensor_add(gsl(0, d2), gsl(0, d2), gsl(HD, d2))
        nc.vector.tensor_add(gsl(2 * HD, d2), gsl(2 * HD, d2), gsl(3 * HD, d2))
        nc.vector.tensor_add(gsl(0, d2), gsl(0, d2), gsl(2 * HD, d2))
        # lp folds: 16 -> 8 -> 4 -> 2 (per head; h stride 16*128)
        for w in (8, 4, 2):
            dh = [[16 * 128, NH], [128, w], [1, HD]]
            nc.vector.tensor_add(gsl(0, dh), gsl(0, dh), gsl(w * 128, dh))
        # final fold writes the fp32 attention output slice layout
        acv = sap(acat[:, :], 0, [[acat[:, :].ap[0][0], P], [HD, NH], [1, HD]])
        dh1 = [[16 * 128, NH], [1, HD]]
        nc.vector.tensor_add(acv, gsl(0, dh1), gsl(128, dh1))

        # ---- output projection + LN + FFN + LN ----
        ac16 = opool.tile([P, EMB], f16, name="ac16", tag="ac16")
        nc.vector.tensor_copy(ac16[:, :], acat[:, :])
        atp = ps_tr.tile([P, 2, P], f16, name="atp", tag="tr")
        nc.tensor.transpose(atp[:, 0, :], ac16[:, 0:P], idf16[:, :])
        nc.tensor.transpose(atp[:, 1, :], ac16[:, P:EMB], idf16[:, :])
        ats = opool.tile([P, 2, P], f16, name="ats", tag="ats")
        nc.vector.tensor_copy(ats[:, :, :], atp[:, :, :])
        oprj = ps_mm.tile([P, EMB], f32, name="oprj", tag="mm")
        mm(oprj, [(ats[:, 0, :], Wout[:, 0, :]),
                  (ats[:, 1, :], Wout[:, 1, :])], bias=bout[:1, :])

        r1 = opool.tile([P, EMB], f32, name="r1", tag="r1")
        nc.vector.tensor_add(r1[:, :], oprj[:, :], fq[:, :])
        x1 = emit_ln(r1, ln1g, ln1b, "la")

        x16 = opool.tile([P, EMB], f16, name="x16", tag="x16")
        nc.vector.tensor_copy(x16[:, :], x1[:, :])
        xtp = ps_tr.tile([P, 2, P], f16, name="xtp", tag="tr")
        nc.tensor.transpose(xtp[:, 0, :], x16[:, 0:P], idf16[:, :])
        nc.tensor.transpose(xtp[:, 1, :], x16[:, P:EMB], idf16[:, :])
        xts = opool.tile([P, 2, P], f16, name="xts", tag="xts")
        nc.vector.tensor_copy(xts[:, :, :], xtp[:, :, :])

        h1s = opool.tile([P, DFFN // P, P], f16, name="h1s", tag="h1s")
        hp = ps_mm.tile([P, DFFN // P, P], f32, name="hp", tag="hpw", bufs=1)
        for mt in range(DFFN // P):
            nc.tensor.matmul(hp[:, mt, :], W1[:, 0, mt * P:(mt + 1) * P],
                             xts[:, 0, :], start=True, stop=False)
            nc.tensor.matmul(hp[:, mt, :], W1[:, 1, mt * P:(mt + 1) * P],
                             xts[:, 1, :], start=False, stop=False)
            nc.tensor.matmul(hp[:, mt, :], b1r[:1, mt * P:(mt + 1) * P],
                             onesr[:1, :], start=False, stop=True)
        nc.scalar.activation(h1s[:, :, :], hp[:, :, :], act_f.Relu)

        yp = ps_mm.tile([P, EMB], f32, name="yp", tag="mm")
        for mt in range(DFFN // P):
            nc.tensor.matmul(yp[:, :], h1s[:, mt, :], W2[:, mt, :],
                             start=(mt == 0), stop=False)
        nc.tensor.matmul(yp[:, :], onesr[:1, :], b2r[:1, :],
                         start=False, stop=True)

        r2 = opool.tile([P, EMB], f32, name="r2", tag="r2")
        nc.vector.tensor_add(r2[:, :], yp[:, :], x1[:, :])
        x2 = emit_ln(r2, ln2g, ln2b, "lb")

        # per-row int8 quantization: scale = rowmax/127, shipped alongside
        absx = opool.tile([P, EMB], f32, name="absx", tag="absx")
        nc.scalar.activation(absx[:, :], x2[:, :], act_f.Abs)
        rmax = opool.tile([P, 1], f32, name="rmax2", tag="rmax2")
        nc.vector.reduce_max(rmax[:, :], absx[:, :], axis=AX.X)
        nc.vector.tensor_scalar_max(rmax[:, :], rmax[:, :], 1e-6)
        rinv = opool.tile([P, 1], f32, name="rinv", tag="rinv")
        nc.vector.reciprocal(rinv[:, :], rmax[:, :])
        smul = opool.tile([P, 1], f32, name="smul", tag="smul")
        nc.scalar.mul(smul[:, :], rinv[:, :], 127.0)
        osc = opool.tile([P, 1], f16, name="osc", tag="osc")
        nc.scalar.mul(osc[:, :], rmax[:, :], 1.0 / 127.0)
        # round(x*smul) = floor(x*smul + 0.5), floor robust to cast mode
        tq = opool.tile([P, EMB], f32, name="tq", tag="tq")
        nc.vector.tensor_scalar(tq[:, :], x2[:, :], smul[:, :], 0.5,
                                op0=op.mult, op1=op.add)
        qi = opool.tile([P, EMB], i32, name="qi", tag="qi")
        nc.vector.tensor_copy(qi[:, :], tq[:, :])
        qf = opool.tile([P, EMB], f32, name="qf", tag="qf")
        nc.vector.tensor_copy(qf[:, :], qi[:, :])
        qm = opool.tile([P, EMB], f32, name="qm", tag="qm")
        nc.vector.tensor_tensor(qm[:, :], tq[:, :], qf[:, :], op=op.is_lt)
        nc.vector.tensor_sub(qf[:, :], qf[:, :], qm[:, :])
        x2q = opool.tile([P, EMB], i8, name="x2q", tag="x2q")
        nc.vector.tensor_copy(x2q[:, :], qf[:, :])
        dma(outs["out_q"][blk * P:(blk + 1) * P, :], x2q)
        dma(outs["out_s"][blk * P:(blk + 1) * P, :], osc)

    for g in range(NGRP):
        emit_group(g)

    ctx.close()


# ------------------------------------------------------------ host entry ---

_CACHE = {}


def build_nc(cfg):
    import concourse.bass as bass
    from concourse import bacc, mybir, tile

    nc = bacc.Bacc("TRN2", debug=False, num_devices=8)
    f32 = mybir.dt.float32
    f16 = mybir.dt.float16

    def di(name, shape, dt=None):
        return nc.dram_tensor(name, list(shape), dt or f32,
                              kind="ExternalInput").ap()

    i8 = mybir.dt.int8
    HQ = cfg["HQ"]
    ins = dict(
        feat_h=di("feat_h", [HQ, EMB], i8),
        pos_h=di("pos_h", [HQ, EMB // 2], i8),
        rsc=di("rsc", [HQ, 2]),
        ref_q=di("ref_q", [HQ, NL, 2], f16),
        wchunk=di("wchunk", [WCHUNK], f16),
        sb16=di("sb16", [1, SB16TOT], f16),
        sb32=di("sb32", [1, SB32TOT]),
    )
    outs = dict(
        out_q=nc.dram_tensor("out_q", [HQ, EMB], i8,
                             kind="ExternalOutput").ap(),
        out_s=nc.dram_tensor("out_s", [HQ, 1], f16,
                             kind="ExternalOutput").ap(),
    )
    with tile.TileContext(nc) as tc:
        emit_kernel(tc, outs, ins, cfg)
    nc.compile()
    return nc


def make_in_maps(inputs, cfg):
    feats = np.asarray(inputs["features"], np.float32)
    pos = np.asarray(inputs["pos"], np.float32)
    refp = np.asarray(inputs["reference_points"], np.float32)
    B = feats.shape[0]
    HQ, L = cfg["HQ"], cfg["L"]
    half = L // 2

    consts = host_constants(cfg)
    wblob = np.concatenate(
        [np.asarray(inputs[k], np.float32).astype(np.float16).reshape(-1)
         for k in WORDER])
    assert wblob.size == WTOT

    sb16src = dict(b_val=inputs["b_val"], b_off=inputs["b_off"],
                   b_attn=inputs["b_attn"], b_out=inputs["b_out"],
                   b1=inputs["b1"], b2=inputs["b2"],
                   ones_row=consts["ones_row"], ident=consts["ident"])
    sb16 = np.concatenate(
        [np.asarray(sb16src[n], np.float32).reshape(-1)
         for n, _ in SB16ORD]).astype(np.float16).reshape(1, -1)
    assert sb16.size == SB16TOT
    sb32src = dict(ln1_g=inputs["ln1_g"], ln1_b=inputs["ln1_b"],
                   ln2_g=inputs["ln2_g"], ln2_b=inputs["ln2_b"],
                   cst_xy=consts["cst_xy"], cst_hlp=consts["cst_hlp"])
    sb32 = np.concatenate(
        [np.asarray(sb32src[n], np.float32).reshape(-1)
         for n, _ in SB32ORD]).astype(np.float32).reshape(1, -1)
    assert sb32.size == SB32TOT

    def quant_rows(x, nrow):
        # per-row symmetric int8: q = round(x/scale), scale = rowmax/127
        q = np.zeros((nrow, x.shape[1]), np.int8)
        sc = np.ones((nrow, 1), np.float32)
        mx = np.abs(x).max(axis=1, keepdims=True)
        mx = np.maximum(mx, 1e-12)
        sc[:x.shape[0]] = (mx / 127.0).astype(np.float32)
        q[:x.shape[0]] = np.clip(np.rint(x / (mx / 127.0)), -127, 127
                                 ).astype(np.int8)
        return q, sc

    def quant_pos4(x, nrow):
        # per-row int4 nibbles: v = clip(round(x/s), -7, 7)+8, s = rowmax/7;
        # packed pairs (lo | hi<<4) shipped as biased int8 (byte - 128)
        pk = np.zeros((nrow, x.shape[1] // 2), np.int8)
        sc = np.ones((nrow, 1), np.float32)
        mx = np.maximum(np.abs(x).max(axis=1, keepdims=True), 1e-12)
        s = (mx / 7.0).astype(np.float32)
        q = (np.clip(np.rint(x / s), -7, 7) + 8).astype(np.int16)
        by = q[:, 0::2] + (q[:, 1::2] << 4)
        pk[:x.shape[0]] = (by - 128).astype(np.int8)
        pk[x.shape[0]:] = 8 + (8 << 4) - 128
        sc[:x.shape[0]] = s
        return pk, sc

    halves = [(0, half), (half, L)]
    in_maps = []
    for core in range(2 * B):
        b, hf = core // 2, core % 2
        s, e = halves[hf]
        fh, fsc = quant_rows(feats[b, s:e], HQ)
        ph, psc = quant_pos4(pos[b, s:e], HQ)
        rq = np.zeros((HQ, NL, 2), np.float16)
        rq[:e - s] = refp[b, s:e].astype(np.float16)
        m = dict(feat_h=fh, pos_h=ph, ref_q=rq,
                 rsc=np.ascontiguousarray(np.hstack([fsc, psc])),
                 wchunk=np.ascontiguousarray(
                     wblob[core * WCHUNK:(core + 1) * WCHUNK]),
                 sb16=sb16, sb32=sb32)
        in_maps.append(m)
    return in_maps, halves


def assemble_out(res, B, L, halves):
    out = np.zeros((B, L, EMB), np.float32)
    for core in range(2 * B):
        b, hf = core // 2, core % 2
        s, e = halves[hf]
        n = e - s
        q = res.results[core]["out_q"][:n].astype(np.float32)
        sc = res.results[core]["out_s"][:n]
        out[b, s:e] = q * sc
    return out


def kernel(**inputs):
    from concourse import bass_utils

    cfg = CFG_FULL
    in_maps, halves = make_in_maps(inputs, cfg)
    B = np.asarray(inputs["features"]).shape[0]
    L = cfg["L"]

    if "nc" not in _CACHE:
        _CACHE["nc"] = build_nc(cfg)
    nc = _CACHE["nc"]

    res = bass_utils.run_bass_kernel_spmd(nc, in_maps,
                                          core_ids=list(range(2 * B)))
    return assemble_out(res, B, L, halves)


# revision 40
# speedup vs baseline: 1.4801x; 1.1812x over previous
"""Trainium2 Bass kernel for a Deformable-DETR style encoder block.

Sharding: 8 NeuronCores = 4 batch samples x 2 query-halves.

The dispatch is wire-transfer-bound (axon tunnel), so inputs/outputs are
aggressively compressed and nothing is uploaded twice:
  - features: per-row-scaled int8, own half only; the value projection is
    computed per half and the full per-batch table assembled on-device via
    a pair AllGather (cores 2b <-> 2b+1).
  - pos: per-row-scaled int4 nibble pairs (unpacked arithmetically on DVE);
    reference points fp16.
  - weights: fp16, uploaded sharded 1/8 per core and reassembled with an
    8-way AllGather; all matmuls run fp16 (PSUM accumulates fp32).
  - output: per-row-scaled int8 + fp16 row scales, dequantized on host.

Per core:
  - value projection of own half -> pair AllGather -> fp16 "patch table"
    in DRAM: for cell (y,x) and head h the 2x2 neighborhood [V[y,x],
    V[y,x+1], V[y+1,x], V[y+1,x+1]] is packed contiguously (4*32 fp16 =
    256B), so one dma_gather descriptor fetches a complete bilinear patch.
  - offset/attention projections, softmax, bilinear weights and cell
    indices computed query-major (PE transposes feed the matmuls).
  - gpsimd indirect DMA fetches patches; DVE multiplies and tree-reduces.
  - output projection + LayerNorm + FFN + LayerNorm, int8 quant, DMA out.
"""

import numpy as np
from contextlib import ExitStack

EMB = 256
NH = 8
NL = 4
NPT = 4
HD = 32
DFFN = 1024
P = 128


def make_cfg(shapes, n_blk_q, grp):
    L = sum(h * w for h, w in shapes)
    starts = np.cumsum([0] + [h * w for h, w in shapes])[:-1].tolist()
    n_blk_full = -(-L // P)
    assert n_blk_q % grp == 0
    return dict(
        shapes=[tuple(s) for s in shapes], starts=starts, L=L,
        LPAD=n_blk_full * P, NBF=n_blk_full, NBQ=n_blk_q, HQ=n_blk_q * P,
        GRP=grp, NGRP=n_blk_q // grp,
    )


CFG_FULL = make_cfg([(100, 100), (50, 50), (25, 25), (13, 13)], 52, 1)
HALF = 6647

# weight blob layout: name -> (element offset, k // P, n), fp16 elements
WORDER = ["W_val", "W_off", "W_attn", "W_out", "W1", "W2"]
WSHAPES = {"W_val": (EMB, EMB), "W_off": (EMB, EMB),
           "W_attn": (EMB, NH * NL * NPT), "W_out": (EMB, EMB),
           "W1": (EMB, DFFN), "W2": (DFFN, EMB)}
WOFFS = {}
_off = 0
for _n in WORDER:
    _k, _c = WSHAPES[_n]
    WOFFS[_n] = (_off, _k // P, _c)
    _off += _k * _c
WTOT = _off
assert WTOT % 8 == 0
WCHUNK = WTOT // 8

# packed small-constant blobs (fp16 / fp32), offsets in elements
SB16ORD = [("b_val", EMB), ("b_off", EMB), ("b_attn", NH * NL * NPT),
           ("b_out", EMB), ("b1", DFFN), ("b2", EMB), ("ones_row", P),
           ("ident", P * P)]
SB16OFF = {}
_off = 0
for _n, _c in SB16ORD:
    SB16OFF[_n] = _off
    _off += _c
SB16TOT = _off
SB32ORD = [("ln1_g", EMB), ("ln1_b", EMB), ("ln2_g", EMB), ("ln2_b", EMB),
           ("cst_xy", 4 * EMB), ("cst_hlp", 3 * P)]
SB32OFF = {}
_off = 0
for _n, _c in SB32ORD:
    SB32OFF[_n] = _off
    _off += _c
SB32TOT = _off


# ------------------------------------------------------- host-side consts ---

def host_constants(cfg):
    shapes, starts = cfg["shapes"], cfg["starts"]
    invnorm = np.zeros(EMB, np.float32)
    pixscale = np.zeros(EMB, np.float32)
    clipmax = np.zeros(EMB, np.float32)
    vmax = np.zeros(EMB, np.float32)
    for h in range(NH):
        for l, (H_, W_) in enumerate(shapes):
            for pt in range(NPT):
                base = h * (NL * NPT * 2) + l * (NPT * 2) + pt * 2
                invnorm[base + 0] = 1.0 / W_
                invnorm[base + 1] = 1.0 / H_
                pixscale[base + 0] = W_
                pixscale[base + 1] = H_
                clipmax[base + 0] = W_ - 2
                clipmax[base + 1] = H_ - 2
                vmax[base + 0] = W_ - 1
                vmax[base + 1] = H_ - 1
    cst_xy = np.stack([invnorm, pixscale, clipmax, vmax])

    wrow = np.zeros(P, np.float32)
    srow = np.zeros(P, np.float32)
    hrow = np.zeros(P, np.float32)
    L = cfg["L"]
    for h in range(NH):
        for l, (H_, W_) in enumerate(shapes):
            for pt in range(NPT):
                base = h * (NL * NPT) + l * NPT + pt
                wrow[base] = W_
                srow[base] = starts[l]
                hrow[base] = h * L
    cst_hlp = np.stack([wrow, srow, hrow])

    ident = np.eye(P, dtype=np.float16)
    ones_row = np.ones((1, P), np.float16)
    return dict(cst_xy=cst_xy, cst_hlp=cst_hlp, ident=ident,
                ones_row=ones_row)


# ------------------------------------------------------------- emission ---

def emit_kernel(tc, outs, ins, cfg, gather_mode="loop"):
    import concourse.bass as bass
    from concourse import mybir

    nc = tc.nc
    op = mybir.AluOpType
    act_f = mybir.ActivationFunctionType
    f32, f16 = mybir.dt.float32, mybir.dt.float16
    i32 = mybir.dt.int32
    AX = mybir.AxisListType

    shapes, starts = cfg["shapes"], cfg["starts"]
    L, NBQ, NGRP = (cfg[k] for k in ("L", "NBQ", "NGRP"))

    ctx = ExitStack()

    def dap(handle, offset, dims):
        return bass.AP(tensor=handle, offset=offset,
                       ap=[list(d) for d in dims])

    def sap(ap0, extra_off, dims):
        return bass.AP(tensor=ap0.tensor, offset=ap0.offset + extra_off,
                       ap=[list(d) for d in dims])

    i8 = mybir.dt.int8

    # ---- internal DRAM ----
    val_half = nc.dram_tensor("val_half", [HALF, EMB], f16, kind="Internal")
    val_full = nc.dram_tensor("val_full", [2 * HALF, EMB], f16,
                              kind="Internal")
    tableT = nc.dram_tensor("tableT", [NH * L, 4 * HD], f16, kind="Internal")
    wb_in = nc.dram_tensor("wb_in", [WCHUNK], f16, kind="Internal")
    wblob = nc.dram_tensor("wblob", [8 * WCHUNK], f16, kind="Internal")

    # ---- pools ----
    cpool = ctx.enter_context(tc.tile_pool(name="consts", bufs=1))
    apool = ctx.enter_context(tc.tile_pool(name="acts", bufs=3))
    wpool = ctx.enter_context(tc.tile_pool(name="wmath", bufs=1))
    gpool = ctx.enter_context(tc.tile_pool(name="gath", bufs=2))
    kpool = ctx.enter_context(tc.tile_pool(name="comb", bufs=2))
    opool = ctx.enter_context(tc.tile_pool(name="outp", bufs=2))
    ps_tr = ctx.enter_context(tc.tile_pool(name="ps_tr", bufs=2, space="PSUM"))
    ps_mm = ctx.enter_context(tc.tile_pool(name="ps_mm", bufs=2, space="PSUM"))
    ps_sm = ctx.enter_context(tc.tile_pool(name="ps_sm", bufs=2, space="PSUM"))

    def dma(out_ap, in_ap):
        nc.sync.dma_start(out=out_ap, in_=in_ap)

    # ---- weights: sharded upload, 8-core AllGather, then load from blob ----
    dma(wb_in.ap()[:], ins["wchunk"][:])
    nc.gpsimd.collective_compute(
        "AllGather",
        mybir.AluOpType.bypass,
        replica_groups=[[0, 1, 2, 3, 4, 5, 6, 7]],
        ins=[wb_in.ap()[:]],
        outs=[wblob.ap()[:]],
    )

    def load_w(name):
        base, a, n = WOFFS[name]
        t = cpool.tile([P, a, n], f16, name=f"s_{name}")
        dma(t, dap(wblob, base, [[n, P], [P * n, a], [1, n]]))
        return t

    Wval = load_w("W_val")
    Woff = load_w("W_off")
    Watt = load_w("W_attn")
    Wout = load_w("W_out")
    W1 = load_w("W1")
    W2 = load_w("W2")

    sb16t = ins["sb16"].tensor
    sb32t = ins["sb32"].tensor

    def load_row(name, n):
        t = cpool.tile([1, n], f16, name=f"r_{name}")
        dma(t, dap(sb16t, SB16OFF[name], [[n, 1], [1, n]]))
        return t

    bval = load_row("b_val", EMB)
    boff = load_row("b_off", EMB)
    batt = load_row("b_attn", NH * NL * NPT)
    bout = load_row("b_out", EMB)
    b1r = load_row("b1", DFFN)
    b2r = load_row("b2", EMB)
    onesr = load_row("ones_row", P)

    def load_bc(off, n, name):
        t = cpool.tile([P, n], f32, name=f"b_{name}")
        dma(t, dap(sb32t, off, [[0, P], [1, n]]))
        return t

    ln1g = load_bc(SB32OFF["ln1_g"], EMB, "ln1g")
    ln1b = load_bc(SB32OFF["ln1_b"], EMB, "ln1b")
    ln2g = load_bc(SB32OFF["ln2_g"], EMB, "ln2g")
    ln2b = load_bc(SB32OFF["ln2_b"], EMB, "ln2b")
    c_invn = load_bc(SB32OFF["cst_xy"], EMB, "invn")
    c_pixs = load_bc(SB32OFF["cst_xy"] + EMB, EMB, "pixs")
    c_clip = load_bc(SB32OFF["cst_xy"] + 2 * EMB, EMB, "clip")
    c_vmax = load_bc(SB32OFF["cst_xy"] + 3 * EMB, EMB, "vmax")
    c_W = load_bc(SB32OFF["cst_hlp"], P, "cw")
    c_S = load_bc(SB32OFF["cst_hlp"] + P, P, "cs")
    c_HL = load_bc(SB32OFF["cst_hlp"] + 2 * P, P, "chl")

    idf16 = cpool.tile([P, P], f16, name="idf16")
    dma(idf16, dap(sb16t, SB16OFF["ident"], [[P, P], [1, P]]))
    eps_t = cpool.tile([P, 1], f32, name="eps_t")
    nc.vector.memset(eps_t[:, :], 1e-5)

    refr = cpool.tile([P, NBQ, 2 * NL], f16, name="refr")
    dma(refr, ins["ref_q"].rearrange("(b p) l c -> p b (l c)", p=P))
    rsct = cpool.tile([P, NBQ, 2], f32, name="rsct")
    dma(rsct, ins["rsc"].rearrange("(b p) c -> p b c", p=P))
    m8sall = cpool.tile([P, NBQ], f32, name="m8sall")
    nc.scalar.mul(m8sall[:, :], rsct[:, :, 1], -8.0)

    def mm(psum_ap, pairs, bias=None):
        seq = list(pairs)
        if bias is not None:
            seq.append((onesr[:1, :psum_ap.shape[0]], bias))
        for i, (lt, rt) in enumerate(seq):
            nc.tensor.matmul(psum_ap, lt, rt,
                             start=(i == 0), stop=(i == len(seq) - 1))

    # ============ P1: value projection of the own half ============
    for blk in range(NBQ):
        fi8 = apool.tile([P, EMB], i8, name="fi8", tag="fi8")
        dma(fi8, ins["feat_h"][blk * P:(blk + 1) * P, :])
        fv = apool.tile([P, EMB], f16, name="fv", tag="fv")
        nc.vector.tensor_scalar_mul(fv[:, :], fi8[:, :],
                                    rsct[:, blk, 0:1])
        ftp = ps_tr.tile([P, 2, P], f16, name="ftp", tag="tr")
        nc.tensor.transpose(ftp[:, 0, :], fv[:, 0:P], idf16[:, :])
        nc.tensor.transpose(ftp[:, 1, :], fv[:, P:EMB], idf16[:, :])
        fts = apool.tile([P, 2, P], f16, name="fts", tag="fts")
        nc.vector.tensor_copy(fts[:, :, :], ftp[:, :, :])
        vp = ps_mm.tile([P, EMB], f32, name="vp", tag="mm")
        mm(vp, [(fts[:, 0, :], Wval[:, 0, :]), (fts[:, 1, :], Wval[:, 1, :])],
           bias=bval[:1, :])
        vf = apool.tile([P, EMB], f16, name="vf", tag="vf")
        nc.vector.tensor_copy(vf[:, :], vp[:, :])
        nrow = min(P, HALF - blk * P)
        dma(val_half.ap()[blk * P:blk * P + nrow, :], vf[:nrow, :])

    # ============ pair AllGather -> full value table ============
    nc.gpsimd.collective_compute(
        "AllGather",
        mybir.AluOpType.bypass,
        replica_groups=[[0, 1], [2, 3], [4, 5], [6, 7]],
        ins=[val_half.ap()[:, :]],
        outs=[val_full.ap()[:, :]],
    )

    # ======================= P2: patch-table build ======================
    for h in range(NH):
        for l, (H_, W_) in enumerate(shapes):
            s = starts[l]
            for cy in (0, 1):
                for cx in (0, 1):
                    c = cy * 2 + cx
                    src = dap(val_full, (s + cy * W_ + cx) * EMB + h * HD,
                              [[W_ * EMB, H_ - 1], [EMB, W_ - 1], [1, HD]])
                    dst = dap(tableT, (h * L + s) * 4 * HD + c * HD,
                              [[W_ * 4 * HD, H_ - 1], [4 * HD, W_ - 1],
                               [1, HD]])
                    dma(dst, src)
            # fill never-gathered edge records (x=W-1 col, y=H-1 row) so the
            # table contains no uninitialized (possibly non-finite) bytes
            dma(dap(tableT, (h * L + s + W_ - 1) * 4 * HD,
                    [[W_ * 4 * HD, H_], [HD, 4], [1, HD]]),
                dap(val_full, (s + W_ - 1) * EMB + h * HD,
                    [[W_ * EMB, H_], [0, 4], [1, HD]]))
            dma(dap(tableT, (h * L + s + (H_ - 1) * W_) * 4 * HD,
                    [[4 * HD, W_ - 1], [HD, 4], [1, HD]]),
                dap(val_full, (s + (H_ - 1) * W_) * EMB + h * HD,
                    [[EMB, W_ - 1], [0, 4], [1, HD]]))

    # ==================== per-block frontend ====================
    def emit_frontend(blk):
        fq8 = apool.tile([P, EMB], i8, name="fq8", tag="fq8")
        dma(fq8, ins["feat_h"][blk * P:(blk + 1) * P, :])
        fq = apool.tile([P, EMB], f16, name="fq", tag="fq", bufs=3)
        nc.vector.tensor_scalar_mul(fq[:, :], fq8[:, :],
                                    rsct[:, blk, 0:1])
        # pos arrives as packed int4 nibble pairs (biased byte - 128, int8)
        HB = EMB // 2
        pq4 = apool.tile([P, HB], i8, name="pq4", tag="pq4")
        dma(pq4, ins["pos_h"][blk * P:(blk + 1) * P, :])
        p4f = apool.tile([P, HB], f32, name="p4f", tag="p4f")
        nc.vector.tensor_scalar_add(p4f[:, :], pq4[:, :], 128.0)
        tnib = apool.tile([P, HB], f32, name="tnib", tag="tnib")
        nc.vector.tensor_scalar_mul(tnib[:, :], p4f[:, :], 1.0 / 16.0)
        tni = apool.tile([P, HB], i32, name="tni", tag="tni")
        nc.vector.tensor_copy(tni[:, :], tnib[:, :])
        hi4 = apool.tile([P, HB], f32, name="hi4", tag="hi4")
        nc.vector.tensor_copy(hi4[:, :], tni[:, :])
        mfx = apool.tile([P, HB], f32, name="mfx", tag="mfx")
        nc.vector.tensor_tensor(mfx[:, :], tnib[:, :], hi4[:, :], op=op.is_lt)
        nc.vector.tensor_sub(hi4[:, :], hi4[:, :], mfx[:, :])
        lo4 = apool.tile([P, HB], f32, name="lo4", tag="lo4")
        nc.vector.scalar_tensor_tensor(lo4[:, :], hi4[:, :], -16.0, p4f[:, :],
                                       op0=op.mult, op1=op.add)
        pq = apool.tile([P, EMB], f16, name="pq", tag="pq")
        pstr = pq[:, :].ap[0][0]
        nc.vector.tensor_scalar(sap(pq[:, :], 0, [[pstr, P], [2, HB]]),
                                lo4[:, :], rsct[:, blk, 1:2],
                                m8sall[:, blk:blk + 1],
                                op0=op.mult, op1=op.add)
        nc.vector.tensor_scalar(sap(pq[:, :], 1, [[pstr, P], [2, HB]]),
                                hi4[:, :], rsct[:, blk, 1:2],
                                m8sall[:, blk:blk + 1],
                                op0=op.mult, op1=op.add)
        qb = apool.tile([P, EMB], f16, name="qb", tag="qb")
        nc.vector.tensor_add(qb[:, :], fq[:, :], pq[:, :])

        qtp = ps_tr.tile([P, 2, P], f16, name="qtp", tag="tr")
        nc.tensor.transpose(qtp[:, 0, :], qb[:, 0:P], idf16[:, :])
        nc.tensor.transpose(qtp[:, 1, :], qb[:, P:EMB], idf16[:, :])
        qts = apool.tile([P, 2, P], f16, name="qts", tag="qts", bufs=2)
        nc.vector.tensor_copy(qts[:, :, :], qtp[:, :, :])

        offp = ps_mm.tile([P, EMB], f32, name="offp", tag="mm")
        mm(offp, [(qts[:, 0, :], Woff[:, 0, :]), (qts[:, 1, :], Woff[:, 1, :])],
           bias=boff[:1, :])
        off = wpool.tile([P, EMB], f32, name="off", tag="off")
        nc.vector.tensor_copy(off[:, :], offp[:, :])

        attp = ps_sm.tile([P, NH * 16], f32, name="attp", tag="sm")
        mm(attp, [(qts[:, 0, :], Watt[:, 0, :]), (qts[:, 1, :], Watt[:, 1, :])],
           bias=batt[:1, :])
        att = wpool.tile([P, NH, 16], f32, name="att", tag="att")
        nc.vector.tensor_copy(att[:, :, :], attp[:, :].rearrange(
            "p (h l) -> p h l", h=NH))

        # softmax over (l,pt) per head
        rmax = wpool.tile([P, NH], f32, name="rmax", tag="rmax")
        nc.vector.reduce_max(rmax[:, :], att[:, :, :], axis=AX.X)
        exv = wpool.tile([P, NH, 16], f32, name="exv", tag="exv")
        rmaxa = rmax[:, :]
        nc.vector.tensor_sub(exv[:, :, :], att[:, :, :],
                             sap(rmaxa, 0, [rmaxa.ap[0], [1, NH], [0, 16]]))
        nc.scalar.activation(exv[:, :, :], exv[:, :, :], act_f.Exp)
        ssum = wpool.tile([P, NH], f32, name="ssum", tag="ssum")
        nc.vector.reduce_sum(ssum[:, :], exv[:, :, :], axis=AX.X)
        rsum = wpool.tile([P, NH], f32, name="rsum", tag="rsum")
        nc.vector.reciprocal(rsum[:, :], ssum[:, :])
        aw = wpool.tile([P, NH, 16], f32, name="aw", tag="aw")
        rsuma = rsum[:, :]
        nc.vector.tensor_mul(aw[:, :, :], exv[:, :, :],
                             sap(rsuma, 0, [rsuma.ap[0], [1, NH], [0, 16]]))

        def wt(name):
            return wpool.tile([P, EMB], f32, name=name, tag=name)

        loc = wt("loc")
        nc.vector.tensor_mul(loc[:, :], off[:, :], c_invn[:, :])
        refa = refr[:, blk, :]
        for xy in (0, 1):
            lvh = sap(loc[:, :], xy, [loc[:, :].ap[0], [32, NH], [8, NL],
                                      [2, NPT]])
            nc.vector.tensor_add(lvh, lvh,
                                 sap(refa, xy, [refa.ap[0], [0, NH], [2, NL],
                                                [0, NPT]]))
        pix = wt("pix")
        nc.vector.tensor_mul(pix[:, :], loc[:, :], c_pixs[:, :])
        nc.vector.tensor_scalar_add(pix[:, :], pix[:, :], -0.5)

        # floor(pix) robust to cast rounding mode
        xi = wpool.tile([P, EMB], i32, name="xi", tag="xi")
        nc.vector.tensor_copy(xi[:, :], pix[:, :])
        base = wt("base")
        nc.vector.tensor_copy(base[:, :], xi[:, :])
        fixm = wt("fixm")
        nc.vector.tensor_tensor(fixm[:, :], pix[:, :], base[:, :], op=op.is_lt)
        nc.vector.tensor_sub(base[:, :], base[:, :], fixm[:, :])
        wfrac = wt("wfrac")
        nc.vector.tensor_sub(wfrac[:, :], pix[:, :], base[:, :])

        basec = wt("basec")
        nc.vector.tensor_scalar_max(basec[:, :], base[:, :], 0.0)
        nc.vector.tensor_tensor(basec[:, :], basec[:, :], c_clip[:, :],
                                op=op.min)

        v0b = wt("v0b")
        nc.vector.tensor_tensor(v0b[:, :], base[:, :], c_vmax[:, :],
                                op=op.is_le)
        vld0 = wt("vld0")
        nc.vector.scalar_tensor_tensor(vld0[:, :], base[:, :], 0.0, v0b[:, :],
                                       op0=op.is_ge, op1=op.mult)
        v1b = wt("v1b")
        nc.vector.tensor_tensor(v1b[:, :], base[:, :], c_clip[:, :],
                                op=op.is_le)
        vld1 = wt("vld1")
        nc.vector.scalar_tensor_tensor(vld1[:, :], base[:, :], -1.0, v1b[:, :],
                                       op0=op.is_ge, op1=op.mult)

        tsh = wt("tsh")
        nc.vector.tensor_sub(tsh[:, :], base[:, :], basec[:, :])
        e0 = wt("e0")
        nc.vector.tensor_scalar(e0[:, :], tsh[:, :], 0.0, None,
                                op0=op.is_equal)
        em1 = wt("em1")
        nc.vector.tensor_scalar(em1[:, :], tsh[:, :], -1.0, None,
                                op0=op.is_equal)
        ep1 = wt("ep1")
        nc.vector.tensor_scalar(ep1[:, :], tsh[:, :], 1.0, None,
                                op0=op.is_equal)

        u0 = wt("u0")
        nc.vector.tensor_scalar(u0[:, :], wfrac[:, :], -1.0, 1.0, op0=op.mult,
                                op1=op.add)
        nc.vector.tensor_mul(u0[:, :], u0[:, :], vld0[:, :])
        u1 = wt("u1")
        nc.vector.tensor_mul(u1[:, :], wfrac[:, :], vld1[:, :])

        a0 = wt("a0")
        nc.vector.tensor_mul(a0[:, :], u0[:, :], e0[:, :])
        t1 = wt("t1")
        nc.vector.tensor_mul(t1[:, :], u1[:, :], em1[:, :])
        nc.vector.tensor_add(a0[:, :], a0[:, :], t1[:, :])
        a1 = wt("a1")
        nc.vector.tensor_mul(a1[:, :], u0[:, :], ep1[:, :])
        nc.vector.tensor_mul(t1[:, :], u1[:, :], e0[:, :])
        nc.vector.tensor_add(a1[:, :], a1[:, :], t1[:, :])

        def ycols(t):
            return sap(t[:, :], 1, [[t[:, :].ap[0][0], P], [2, P]])

        def xcols(t):
            return sap(t[:, :], 0, [[t[:, :].ap[0][0], P], [2, P]])

        awf = aw.rearrange("p h l -> p (h l)")
        ay0 = wpool.tile([P, P], f32, name="ay0", tag="ay0")
        nc.vector.tensor_mul(ay0[:, :], ycols(a0), awf)
        ay1 = wpool.tile([P, P], f32, name="ay1", tag="ay1")
        nc.vector.tensor_mul(ay1[:, :], ycols(a1), awf)

        w4 = wpool.tile([P, P, 4], f16, name="w4", tag="w4", bufs=2)
        nc.vector.tensor_mul(w4[:, :, 0], ay0[:, :], xcols(a0))
        nc.vector.tensor_mul(w4[:, :, 1], ay0[:, :], xcols(a1))
        nc.vector.tensor_mul(w4[:, :, 2], ay1[:, :], xcols(a0))
        nc.vector.tensor_mul(w4[:, :, 3], ay1[:, :], xcols(a1))

        cell = wpool.tile([P, P], f32, name="cell", tag="cell")
        nc.vector.tensor_mul(cell[:, :], ycols(basec), c_W[:, :])
        nc.vector.tensor_add(cell[:, :], cell[:, :], xcols(basec))
        nc.vector.tensor_add(cell[:, :], cell[:, :], c_S[:, :])

        nc.vector.tensor_add(cell[:, :], cell[:, :], c_HL[:, :])
        offs = wpool.tile([P, P], i32, name="offs", tag="offs", bufs=2)
        nc.vector.tensor_copy(offs[:, :], cell[:, :])
        return fq, w4, offs

    # ==================== LayerNorm ====================
    def emit_ln(r, gt, bt, pfx):
        nsum = opool.tile([P, 1], f32, name=f"{pfx}ns", tag=f"{pfx}ns")
        nc.vector.tensor_reduce(nsum[:, :], r[:, :], axis=AX.X, op=op.add,
                                negate=True)
        nmean = opool.tile([P, 1], f32, name=f"{pfx}nm", tag=f"{pfx}nm")
        nc.scalar.mul(nmean[:, :], nsum[:, :], 1.0 / EMB)
        c = opool.tile([P, EMB], f32, name=f"{pfx}c", tag=f"{pfx}c")
        nc.vector.tensor_scalar_add(c[:, :], r[:, :], nmean[:, :])
        csq = opool.tile([P, EMB], f32, name=f"{pfx}sq", tag=f"{pfx}sq")
        ssq = opool.tile([P, 1], f32, name=f"{pfx}ssq", tag=f"{pfx}ssq")
        nc.scalar.activation(csq[:, :], c[:, :], act_f.Square,
                             accum_out=ssq[:, :])
        std = opool.tile([P, 1], f32, name=f"{pfx}std", tag=f"{pfx}std")
        nc.scalar.activation(std[:, :], ssq[:, :], act_f.Sqrt,
                             bias=eps_t[:, :], scale=1.0 / EMB)
        rstd = opool.tile([P, 1], f32, name=f"{pfx}rs", tag=f"{pfx}rs")
        nc.vector.reciprocal(rstd[:, :], std[:, :])
        x = opool.tile([P, EMB], f32, name=f"{pfx}x", tag=f"{pfx}x")
        nc.vector.scalar_tensor_tensor(x[:, :], c[:, :], rstd[:, :], gt[:, :],
                                       op0=op.mult, op1=op.mult)
        nc.vector.tensor_add(x[:, :], x[:, :], bt[:, :])
        return x

    # ==================== per-group pipeline ====================
    def emit_group(g):
        blk = g
        fq, w4, offs = emit_frontend(blk)
        gb = gpool.tile([P, P, 4 * HD], f16, name="gb", tag="gb", bufs=2)
        if gather_mode == "batched":
            nc.gpsimd.indirect_dma_start(
                out=gb[:, :, :], out_offset=None,
                in_=tableT.ap()[:, :],
                in_offset=bass.IndirectOffsetOnAxis(ap=offs[:, :], axis=0))
        elif gather_mode == "loop":
            for s in range(P):
                nc.gpsimd.indirect_dma_start(
                    out=gb[:, s, :], out_offset=None,
                    in_=tableT.ap()[:, :],
                    in_offset=bass.IndirectOffsetOnAxis(ap=offs[:, s:s + 1],
                                                        axis=0))
        # gather_mode == "skip": timing-ablation only, gb stays uninitialized

        acat = kpool.tile([P, EMB], f32, name="acat", tag="acat")
        # all-heads combine, reduction tree folded in place inside gb
        gba = gb[:, :, :]
        pstr = gba.ap[0][0]

        def gsl(off, dims):
            return sap(gba, off, [[pstr, P]] + dims)

        # weights: w4 [P, (h,lp), 4] broadcast over head_dim (0-stride)
        w4b = sap(w4[:, :, :], 0,
                  [[w4[:, :, :].ap[0][0], P], [4, P], [1, 4], [0, HD]])
        gall = gsl(0, [[128, P], [HD, 4], [1, HD]])
        nc.vector.tensor_mul(gall, gall, w4b)
        # corner folds: c0+=c1, c2+=c3, c0+=c2
        d2 = [[128, P], [1, HD]]
        nc.vector.tensor_add(gsl(0, d2), gsl(0, d2), gsl(HD, d2))
        nc.vector.tensor_add(gsl(2 * HD, d2), gsl(2 * HD, d2), gsl(3 * HD, d2))
        nc.vector.tensor_add(gsl(0, d2), gsl(0, d2), gsl(2 * HD, d2))
        # lp folds: 16 -> 8 -> 4 -> 2 (per head; h stride 16*128)
        for w in (8, 4, 2):
            dh = [[16 * 128, NH], [128, w], [1, HD]]
            nc.vector.tensor_add(gsl(0, dh), gsl(0, dh), gsl(w * 128, dh))
        # final fold writes the fp32 attention output slice layout
        acv = sap(acat[:, :], 0, [[acat[:, :].ap[0][0], P], [HD, NH], [1, HD]])
        dh1 = [[16 * 128, NH], [1, HD]]
        nc.vector.tensor_add(acv, gsl(0, dh1), gsl(128, dh1))

        # ---- output projection + LN + FFN + LN ----
        ac16 = opool.tile([P, EMB], f16, name="ac16", tag="ac16")
        nc.vector.tensor_copy(ac16[:, :], acat[:, :])
        atp = ps_tr.tile([P, 2, P], f16, name="atp", tag="tr")
        nc.tensor.transpose(atp[:, 0, :], ac16[:, 0:P], idf16[:, :])
        nc.tensor.transpose(atp[:, 1, :], ac16[:, P:EMB], idf16[:, :])
        ats = opool.tile([P, 2, P], f16, name="ats", tag="ats")
        nc.vector.tensor_copy(ats[:, :, :], atp[:, :, :])
        oprj = ps_mm.tile([P, EMB], f32, name="oprj", tag="mm")
        mm(oprj, [(ats[:, 0, :], Wout[:, 0, :]),
                  (ats[:, 1, :], Wout[:, 1, :])], bias=bout[:1, :])

        r1 = opool.tile([P, EMB], f32, name="r1", tag="r1")
        nc.vector.tensor_add(r1[:, :], oprj[:, :], fq[:, :])
        x1 = emit_ln(r1, ln1g, ln1b, "la")

        x16 = opool.tile([P, EMB], f16, name="x16", tag="x16")
        nc.vector.tensor_copy(x16[:, :], x1[:, :])
        xtp = ps_tr.tile([P, 2, P], f16, name="xtp", tag="tr")
        nc.tensor.transpose(xtp[:, 0, :], x16[:, 0:P], idf16[:, :])
        nc.tensor.transpose(xtp[:, 1, :], x16[:, P:EMB], idf16[:, :])
        xts = opool.tile([P, 2, P], f16, name="xts", tag="xts")
        nc.vector.tensor_copy(xts[:, :, :], xtp[:, :, :])

        h1s = opool.tile([P, DFFN // P, P], f16, name="h1s", tag="h1s")
        hp = ps_mm.tile([P, DFFN // P, P], f32, name="hp", tag="hpw", bufs=1)
        for mt in range(DFFN // P):
            nc.tensor.matmul(hp[:, mt, :], W1[:, 0, mt * P:(mt + 1) * P],
                             xts[:, 0, :], start=True, stop=False)
            nc.tensor.matmul(hp[:, mt, :], W1[:, 1, mt * P:(mt + 1) * P],
                             xts[:, 1, :], start=False, stop=False)
            nc.tensor.matmul(hp[:, mt, :], b1r[:1, mt * P:(mt + 1) * P],
                             onesr[:1, :], start=False, stop=True)
        nc.scalar.activation(h1s[:, :, :], hp[:, :, :], act_f.Relu)

        yp = ps_mm.tile([P, EMB], f32, name="yp", tag="mm")
        for mt in range(DFFN // P):
            nc.tensor.matmul(yp[:, :], h1s[:, mt, :], W2[:, mt, :],
                             start=(mt == 0), stop=False)
        nc.tensor.matmul(yp[:, :], onesr[:1, :], b2r[:1, :],
                         start=False, stop=True)

        r2 = opool.tile([P, EMB], f32, name="r2", tag="r2")
        nc.vector.tensor_add(r2[:, :], yp[:, :], x1[:, :])
        x2 = emit_ln(r2, ln2g, ln2b, "lb")

        # per-row int8 quantization: scale = rowmax/127, shipped alongside
        absx = opool.tile([P, EMB], f32, name="absx", tag="absx")
        nc.scalar.activation(absx[:, :], x2[:, :], act_f.Abs)
        rmax = opool.tile([P, 1], f32, name="rmax2", tag="rmax2")
        nc.vector.reduce_max(rmax[:, :], absx[:, :], axis=AX.X)
        nc.vector.tensor_scalar_max(rmax[:, :], rmax[:, :], 1e-6)
        rinv = opool.tile([P, 1], f32, name="rinv", tag="rinv")
        nc.vector.reciprocal(rinv[:, :], rmax[:, :])
        smul = opool.tile([P, 1], f32, name="smul", tag="smul")
        nc.scalar.mul(smul[:, :], rinv[:, :], 127.0)
        osc = opool.tile([P, 1], f16, name="osc", tag="osc")
        nc.scalar.mul(osc[:, :], rmax[:, :], 1.0 / 127.0)
        # round(x*smul) = floor(x*smul + 0.5), floor robust to cast mode
        tq = opool.tile([P, EMB], f32, name="tq", tag="tq")
        nc.vector.tensor_scalar(tq[:, :], x2[:, :], smul[:, :], 0.5,
                                op0=op.mult, op1=op.add)
        qi = opool.tile([P, EMB], i32, name="qi", tag="qi")
        nc.vector.tensor_copy(qi[:, :], tq[:, :])
        qf = opool.tile([P, EMB], f32, name="qf", tag="qf")
        nc.vector.tensor_copy(qf[:, :], qi[:, :])
        qm = opool.tile([P, EMB], f32, name="qm", tag="qm")
        nc.vector.tensor_tensor(qm[:, :], tq[:, :], qf[:, :], op=op.is_lt)
        nc.vector.tensor_sub(qf[:, :], qf[:, :], qm[:, :])
        x2q = opool.tile([P, EMB], i8, name="x2q", tag="x2q")
        nc.vector.tensor_copy(x2q[:, :], qf[:, :])
        dma(outs["out_q"][blk * P:(blk + 1) * P, :], x2q)
        dma(outs["out_s"][blk * P:(blk + 1) * P, :], osc)

    for g in range(NGRP):
        emit_group(g)

    ctx.close()


# ------------------------------------------------------------ host entry ---

_CACHE = {}


def build_nc(cfg, gather_mode="loop"):
    import concourse.bass as bass
    from concourse import bacc, mybir, tile

    nc = bacc.Bacc("TRN2", debug=False, num_devices=8)
    f32 = mybir.dt.float32
    f16 = mybir.dt.float16

    def di(name, shape, dt=None):
        return nc.dram_tensor(name, list(shape), dt or f32,
                              kind="ExternalInput").ap()

    i8 = mybir.dt.int8
    HQ = cfg["HQ"]
    ins = dict(
        feat_h=di("feat_h", [HQ, EMB], i8),
        pos_h=di("pos_h", [HQ, EMB // 2], i8),
        rsc=di("rsc", [HQ, 2]),
        ref_q=di("ref_q", [HQ, NL, 2], f16),
        wchunk=di("wchunk", [WCHUNK], f16),
        sb16=di("sb16", [1, SB16TOT], f16),
        sb32=di("sb32", [1, SB32TOT]),
    )
    outs = dict(
        out_q=nc.dram_tensor("out_q", [HQ, EMB], i8,
                             kind="ExternalOutput").ap(),
        out_s=nc.dram_tensor("out_s", [HQ, 1], f16,
                             kind="ExternalOutput").ap(),
    )
    with tile.TileContext(nc) as tc:
        emit_kernel(tc, outs, ins, cfg, gather_mode)
    nc.compile()
    return nc


def make_in_maps(inputs, cfg):
    feats = np.asarray(inputs["features"], np.float32)
    pos = np.asarray(inputs["pos"], np.float32)
    refp = np.asarray(inputs["reference_points"], np.float32)
    B = feats.shape[0]
    HQ, L = cfg["HQ"], cfg["L"]
    half = L // 2

    consts = host_constants(cfg)
    wblob = np.concatenate(
        [np.asarray(inputs[k], np.float32).astype(np.float16).reshape(-1)
         for k in WORDER])
    assert wblob.size == WTOT

    sb16src = dict(b_val=inputs["b_val"], b_off=inputs["b_off"],
                   b_attn=inputs["b_attn"], b_out=inputs["b_out"],
                   b1=inputs["b1"], b2=inputs["b2"],
                   ones_row=consts["ones_row"], ident=consts["ident"])
    sb16 = np.concatenate(
        [np.asarray(sb16src[n], np.float32).reshape(-1)
         for n, _ in SB16ORD]).astype(np.float16).reshape(1, -1)
    assert sb16.size == SB16TOT
    sb32src = dict(ln1_g=inputs["ln1_g"], ln1_b=inputs["ln1_b"],
                   ln2_g=inputs["ln2_g"], ln2_b=inputs["ln2_b"],
                   cst_xy=consts["cst_xy"], cst_hlp=consts["cst_hlp"])
    sb32 = np.concatenate(
        [np.asarray(sb32src[n], np.float32).reshape(-1)
         for n, _ in SB32ORD]).astype(np.float32).reshape(1, -1)
    assert sb32.size == SB32TOT

    def quant_rows(x, nrow):
        # per-row symmetric int8: q = round(x/scale), scale = rowmax/127
        q = np.zeros((nrow, x.shape[1]), np.int8)
        sc = np.ones((nrow, 1), np.float32)
        mx = np.abs(x).max(axis=1, keepdims=True)
        mx = np.maximum(mx, 1e-12)
        sc[:x.shape[0]] = (mx / 127.0).astype(np.float32)
        q[:x.shape[0]] = np.clip(np.rint(x / (mx / 127.0)), -127, 127
                                 ).astype(np.int8)
        return q, sc

    def quant_pos4(x, nrow):
        # per-row int4 nibbles: v = clip(round(x/s), -7, 7)+8, s = rowmax/7;
        # packed pairs (lo | hi<<4) shipped as biased int8 (byte - 128)
        pk = np.zeros((nrow, x.shape[1] // 2), np.int8)
        sc = np.ones((nrow, 1), np.float32)
        mx = np.maximum(np.abs(x).max(axis=1, keepdims=True), 1e-12)
        s = (mx / 7.0).astype(np.float32)
        q = (np.clip(np.rint(x / s), -7, 7) + 8).astype(np.int16)
        by = q[:, 0::2] + (q[:, 1::2] << 4)
        pk[:x.shape[0]] = (by - 128).astype(np.int8)
        pk[x.shape[0]:] = 8 + (8 << 4) - 128
        sc[:x.shape[0]] = s
        return pk, sc

    halves = [(0, half), (half, L)]
    in_maps = []
    for core in range(2 * B):
        b, hf = core // 2, core % 2
        s, e = halves[hf]
        fh, fsc = quant_rows(feats[b, s:e], HQ)
        ph, psc = quant_pos4(pos[b, s:e], HQ)
        rq = np.zeros((HQ, NL, 2), np.float16)
        rq[:e - s] = refp[b, s:e].astype(np.float16)
        m = dict(feat_h=fh, pos_h=ph, ref_q=rq,
                 rsc=np.ascontiguousarray(np.hstack([fsc, psc])),
                 wchunk=np.ascontiguousarray(
                     wblob[core * WCHUNK:(core + 1) * WCHUNK]),
                 sb16=sb16, sb32=sb32)
        in_maps.append(m)
    return in_maps, halves


def assemble_out(res, B, L, halves):
    out = np.zeros((B, L, EMB), np.float32)
    for core in range(2 * B):
        b, hf = core // 2, core % 2
        s, e = halves[hf]
        n = e - s
        q = res.results[core]["out_q"][:n].astype(np.float32)
        sc = res.results[core]["out_s"][:n]
        out[b, s:e] = q * sc
    return out


def kernel(**inputs):
    from concourse import bass_utils

    cfg = CFG_FULL
    in_maps, halves = make_in_maps(inputs, cfg)
    B = np.asarray(inputs["features"]).shape[0]
    L = cfg["L"]

    if "nc" not in _CACHE:
        _CACHE["nc"] = build_nc(cfg)
    nc = _CACHE["nc"]

    res = bass_utils.run_bass_kernel_spmd(nc, in_maps,
                                          core_ids=list(range(2 * B)))
    return assemble_out(res, B, L, halves)


# revision 48
# speedup vs baseline: 1.9992x; 1.3507x over previous
"""Trainium2 Bass kernel for a Deformable-DETR style encoder block.

Sharding: 8 NeuronCores = 4 batch samples x 2 query-halves.

The dispatch is wire-transfer-bound (axon tunnel), so inputs/outputs are
aggressively compressed and nothing is uploaded twice:
  - features: per-row-scaled int8, own half only; the value projection is
    computed per half and the full per-batch table assembled on-device via
    a pair AllGather (cores 2b <-> 2b+1).
  - pos: per-row-scaled int4 nibble pairs (unpacked arithmetically on DVE);
    reference points fp16.
  - weights: fp16, uploaded sharded 1/8 per core and reassembled with an
    8-way AllGather; all matmuls run fp16 (PSUM accumulates fp32).
  - output: per-row-scaled int8 + fp16 row scales, dequantized on host.

Per core:
  - value projection of own half -> pair AllGather -> fp16 "patch table"
    in DRAM: for cell (y,x) and head h the 2x2 neighborhood [V[y,x],
    V[y,x+1], V[y+1,x], V[y+1,x+1]] is packed contiguously (4*32 fp16 =
    256B), so one dma_gather descriptor fetches a complete bilinear patch.
  - offset/attention projections, softmax, bilinear weights and cell
    indices computed query-major (PE transposes feed the matmuls).
  - gpsimd indirect DMA fetches patches; DVE multiplies and tree-reduces.
  - output projection + LayerNorm + FFN + LayerNorm, int8 quant, DMA out.
"""

import numpy as np
from contextlib import ExitStack

EMB = 256
NH = 8
NL = 4
NPT = 4
HD = 32
DFFN = 1024
P = 128


def make_cfg(shapes, n_blk_q, grp):
    L = sum(h * w for h, w in shapes)
    starts = np.cumsum([0] + [h * w for h, w in shapes])[:-1].tolist()
    n_blk_full = -(-L // P)
    assert n_blk_q % grp == 0
    return dict(
        shapes=[tuple(s) for s in shapes], starts=starts, L=L,
        LPAD=n_blk_full * P, NBF=n_blk_full, NBQ=n_blk_q, HQ=n_blk_q * P,
        GRP=grp, NGRP=n_blk_q // grp,
    )


CFG_FULL = make_cfg([(100, 100), (50, 50), (25, 25), (13, 13)], 52, 1)
HALF = 6647

# weight blob layout: name -> (element offset, k // P, n), fp16 elements
WORDER = ["W_val", "W_off", "W_attn", "W_out", "W1", "W2"]
WSHAPES = {"W_val": (EMB, EMB), "W_off": (EMB, EMB),
           "W_attn": (EMB, NH * NL * NPT), "W_out": (EMB, EMB),
           "W1": (EMB, DFFN), "W2": (DFFN, EMB)}
WOFFS = {}
_off = 0
for _n in WORDER:
    _k, _c = WSHAPES[_n]
    WOFFS[_n] = (_off, _k // P, _c)
    _off += _k * _c
WTOT = _off
assert WTOT % 8 == 0
WCHUNK = WTOT // 8

# packed small-constant blobs (fp16 / fp32), offsets in elements
SB16ORD = [("b_val", EMB), ("b_off", EMB), ("b_attn", NH * NL * NPT),
           ("b_out", EMB), ("b1", DFFN), ("b2", EMB), ("ones_row", P),
           ("ident", P * P)]
SB16OFF = {}
_off = 0
for _n, _c in SB16ORD:
    SB16OFF[_n] = _off
    _off += _c
SB16TOT = _off
SB32ORD = [("ln1_g", EMB), ("ln1_b", EMB), ("ln2_g", EMB), ("ln2_b", EMB),
           ("cst_xy", 4 * EMB), ("cst_hlp", 3 * P)]
SB32OFF = {}
_off = 0
for _n, _c in SB32ORD:
    SB32OFF[_n] = _off
    _off += _c
SB32TOT = _off


# ------------------------------------------------------- host-side consts ---

def host_constants(cfg):
    shapes, starts = cfg["shapes"], cfg["starts"]
    invnorm = np.zeros(EMB, np.float32)
    pixscale = np.zeros(EMB, np.float32)
    clipmax = np.zeros(EMB, np.float32)
    vmax = np.zeros(EMB, np.float32)
    for h in range(NH):
        for l, (H_, W_) in enumerate(shapes):
            for pt in range(NPT):
                base = h * (NL * NPT * 2) + l * (NPT * 2) + pt * 2
                invnorm[base + 0] = 1.0 / W_
                invnorm[base + 1] = 1.0 / H_
                pixscale[base + 0] = W_
                pixscale[base + 1] = H_
                clipmax[base + 0] = W_ - 2
                clipmax[base + 1] = H_ - 2
                vmax[base + 0] = W_ - 1
                vmax[base + 1] = H_ - 1
    cst_xy = np.stack([invnorm, pixscale, clipmax, vmax])

    wrow = np.zeros(P, np.float32)
    srow = np.zeros(P, np.float32)
    hrow = np.zeros(P, np.float32)
    L = cfg["L"]
    for h in range(NH):
        for l, (H_, W_) in enumerate(shapes):
            for pt in range(NPT):
                base = h * (NL * NPT) + l * NPT + pt
                wrow[base] = W_
                srow[base] = starts[l]
                hrow[base] = h * L
    cst_hlp = np.stack([wrow, srow, hrow])

    ident = np.eye(P, dtype=np.float16)
    ones_row = np.ones((1, P), np.float16)
    return dict(cst_xy=cst_xy, cst_hlp=cst_hlp, ident=ident,
                ones_row=ones_row)


# ------------------------------------------------------------- emission ---

def emit_kernel(tc, outs, ins, cfg, gather_mode="loop"):
    import concourse.bass as bass
    from concourse import mybir

    nc = tc.nc
    op = mybir.AluOpType
    act_f = mybir.ActivationFunctionType
    f32, f16 = mybir.dt.float32, mybir.dt.float16
    i32 = mybir.dt.int32
    AX = mybir.AxisListType

    shapes, starts = cfg["shapes"], cfg["starts"]
    L, NBQ, NGRP = (cfg[k] for k in ("L", "NBQ", "NGRP"))

    ctx = ExitStack()

    def dap(handle, offset, dims):
        return bass.AP(tensor=handle, offset=offset,
                       ap=[list(d) for d in dims])

    def sap(ap0, extra_off, dims):
        return bass.AP(tensor=ap0.tensor, offset=ap0.offset + extra_off,
                       ap=[list(d) for d in dims])

    i8 = mybir.dt.int8

    # ---- internal DRAM ----
    val_half = nc.dram_tensor("val_half", [HALF, EMB], f16, kind="Internal")
    val_full = nc.dram_tensor("val_full", [2 * HALF, EMB], f16,
                              kind="Internal")
    tableT = nc.dram_tensor("tableT", [NH * L, 4 * HD], f16, kind="Internal")
    wb_in = nc.dram_tensor("wb_in", [WCHUNK], f16, kind="Internal")
    wblob = nc.dram_tensor("wblob", [8 * WCHUNK], f16, kind="Internal")
    i16 = mybir.dt.int16
    if gather_mode == "dgather":
        from concourse import library_config
        idxscr = nc.dram_tensor("idxscr", [16, NH * P], i16, kind="Internal")
        nc.gpsimd.load_library(library_config.mlp)

    # ---- pools ----
    cpool = ctx.enter_context(tc.tile_pool(name="consts", bufs=1))
    apool = ctx.enter_context(tc.tile_pool(name="acts", bufs=3))
    wpool = ctx.enter_context(tc.tile_pool(name="wmath", bufs=1))
    gpool = ctx.enter_context(tc.tile_pool(name="gath", bufs=2))
    kpool = ctx.enter_context(tc.tile_pool(name="comb", bufs=2))
    opool = ctx.enter_context(tc.tile_pool(name="outp", bufs=2))
    ps_tr = ctx.enter_context(tc.tile_pool(name="ps_tr", bufs=2, space="PSUM"))
    ps_mm = ctx.enter_context(tc.tile_pool(name="ps_mm", bufs=2, space="PSUM"))
    ps_sm = ctx.enter_context(tc.tile_pool(name="ps_sm", bufs=2, space="PSUM"))

    def dma(out_ap, in_ap):
        nc.sync.dma_start(out=out_ap, in_=in_ap)

    # ---- weights: sharded upload, 8-core AllGather, then load from blob ----
    dma(wb_in.ap()[:], ins["wchunk"][:])
    nc.gpsimd.collective_compute(
        "AllGather",
        mybir.AluOpType.bypass,
        replica_groups=[[0, 1, 2, 3, 4, 5, 6, 7]],
        ins=[wb_in.ap()[:]],
        outs=[wblob.ap()[:]],
    )

    def load_w(name):
        base, a, n = WOFFS[name]
        t = cpool.tile([P, a, n], f16, name=f"s_{name}")
        dma(t, dap(wblob, base, [[n, P], [P * n, a], [1, n]]))
        return t

    Wval = load_w("W_val")
    Woff = load_w("W_off")
    Watt = load_w("W_attn")
    Wout = load_w("W_out")
    W1 = load_w("W1")
    W2 = load_w("W2")

    sb16t = ins["sb16"].tensor
    sb32t = ins["sb32"].tensor

    def load_row(name, n):
        t = cpool.tile([1, n], f16, name=f"r_{name}")
        dma(t, dap(sb16t, SB16OFF[name], [[n, 1], [1, n]]))
        return t

    bval = load_row("b_val", EMB)
    boff = load_row("b_off", EMB)
    batt = load_row("b_attn", NH * NL * NPT)
    bout = load_row("b_out", EMB)
    b1r = load_row("b1", DFFN)
    b2r = load_row("b2", EMB)
    onesr = load_row("ones_row", P)

    def load_bc(off, n, name):
        t = cpool.tile([P, n], f32, name=f"b_{name}")
        dma(t, dap(sb32t, off, [[0, P], [1, n]]))
        return t

    ln1g = load_bc(SB32OFF["ln1_g"], EMB, "ln1g")
    ln1b = load_bc(SB32OFF["ln1_b"], EMB, "ln1b")
    ln2g = load_bc(SB32OFF["ln2_g"], EMB, "ln2g")
    ln2b = load_bc(SB32OFF["ln2_b"], EMB, "ln2b")
    c_invn = load_bc(SB32OFF["cst_xy"], EMB, "invn")
    c_pixs = load_bc(SB32OFF["cst_xy"] + EMB, EMB, "pixs")
    c_clip = load_bc(SB32OFF["cst_xy"] + 2 * EMB, EMB, "clip")
    c_vmax = load_bc(SB32OFF["cst_xy"] + 3 * EMB, EMB, "vmax")
    c_W = load_bc(SB32OFF["cst_hlp"], P, "cw")
    c_S = load_bc(SB32OFF["cst_hlp"] + P, P, "cs")
    c_HL = load_bc(SB32OFF["cst_hlp"] + 2 * P, P, "chl")

    idf16 = cpool.tile([P, P], f16, name="idf16")
    dma(idf16, dap(sb16t, SB16OFF["ident"], [[P, P], [1, P]]))
    eps_t = cpool.tile([P, 1], f32, name="eps_t")
    nc.vector.memset(eps_t[:, :], 1e-5)

    refr = cpool.tile([P, NBQ, 2 * NL], f16, name="refr")
    dma(refr, ins["ref_q"].rearrange("(b p) l c -> p b (l c)", p=P))
    rsct = cpool.tile([P, NBQ, 2], f32, name="rsct")
    dma(rsct, ins["rsc"].rearrange("(b p) c -> p b c", p=P))
    m8sall = cpool.tile([P, NBQ], f32, name="m8sall")
    nc.scalar.mul(m8sall[:, :], rsct[:, :, 1], -8.0)

    def mm(psum_ap, pairs, bias=None):
        seq = list(pairs)
        if bias is not None:
            seq.append((onesr[:1, :psum_ap.shape[0]], bias))
        for i, (lt, rt) in enumerate(seq):
            nc.tensor.matmul(psum_ap, lt, rt,
                             start=(i == 0), stop=(i == len(seq) - 1))

    # ============ P1: value projection of the own half ============
    for blk in range(NBQ):
        fi8 = apool.tile([P, EMB], i8, name="fi8", tag="fi8")
        dma(fi8, ins["feat_h"][blk * P:(blk + 1) * P, :])
        fv = apool.tile([P, EMB], f16, name="fv", tag="fv")
        nc.vector.tensor_scalar_mul(fv[:, :], fi8[:, :],
                                    rsct[:, blk, 0:1])
        ftp = ps_tr.tile([P, 2, P], f16, name="ftp", tag="tr")
        nc.tensor.transpose(ftp[:, 0, :], fv[:, 0:P], idf16[:, :])
        nc.tensor.transpose(ftp[:, 1, :], fv[:, P:EMB], idf16[:, :])
        fts = apool.tile([P, 2, P], f16, name="fts", tag="fts")
        nc.vector.tensor_copy(fts[:, :, :], ftp[:, :, :])
        vp = ps_mm.tile([P, EMB], f32, name="vp", tag="mm")
        mm(vp, [(fts[:, 0, :], Wval[:, 0, :]), (fts[:, 1, :], Wval[:, 1, :])],
           bias=bval[:1, :])
        vf = apool.tile([P, EMB], f16, name="vf", tag="vf")
        nc.vector.tensor_copy(vf[:, :], vp[:, :])
        nrow = min(P, HALF - blk * P)
        dma(val_half.ap()[blk * P:blk * P + nrow, :], vf[:nrow, :])

    # ============ pair AllGather -> full value table ============
    nc.gpsimd.collective_compute(
        "AllGather",
        mybir.AluOpType.bypass,
        replica_groups=[[0, 1], [2, 3], [4, 5], [6, 7]],
        ins=[val_half.ap()[:, :]],
        outs=[val_full.ap()[:, :]],
    )

    # ======================= P2: patch-table build ======================
    for h in range(NH):
        for l, (H_, W_) in enumerate(shapes):
            s = starts[l]
            for cy in (0, 1):
                for cx in (0, 1):
                    c = cy * 2 + cx
                    src = dap(val_full, (s + cy * W_ + cx) * EMB + h * HD,
                              [[W_ * EMB, H_ - 1], [EMB, W_ - 1], [1, HD]])
                    dst = dap(tableT, (h * L + s) * 4 * HD + c * HD,
                              [[W_ * 4 * HD, H_ - 1], [4 * HD, W_ - 1],
                               [1, HD]])
                    dma(dst, src)
            # fill never-gathered edge records (x=W-1 col, y=H-1 row) so the
            # table contains no uninitialized (possibly non-finite) bytes
            dma(dap(tableT, (h * L + s + W_ - 1) * 4 * HD,
                    [[W_ * 4 * HD, H_], [HD, 4], [1, HD]]),
                dap(val_full, (s + W_ - 1) * EMB + h * HD,
                    [[W_ * EMB, H_], [0, 4], [1, HD]]))
            dma(dap(tableT, (h * L + s + (H_ - 1) * W_) * 4 * HD,
                    [[4 * HD, W_ - 1], [HD, 4], [1, HD]]),
                dap(val_full, (s + (H_ - 1) * W_) * EMB + h * HD,
                    [[EMB, W_ - 1], [0, 4], [1, HD]]))

    # ==================== per-block frontend ====================
    def emit_frontend(blk):
        fq8 = apool.tile([P, EMB], i8, name="fq8", tag="fq8")
        dma(fq8, ins["feat_h"][blk * P:(blk + 1) * P, :])
        fq = apool.tile([P, EMB], f16, name="fq", tag="fq", bufs=3)
        nc.vector.tensor_scalar_mul(fq[:, :], fq8[:, :],
                                    rsct[:, blk, 0:1])
        # pos arrives as packed int4 nibble pairs (biased byte - 128, int8)
        HB = EMB // 2
        pq4 = apool.tile([P, HB], i8, name="pq4", tag="pq4")
        dma(pq4, ins["pos_h"][blk * P:(blk + 1) * P, :])
        p4f = apool.tile([P, HB], f32, name="p4f", tag="p4f")
        nc.vector.tensor_scalar_add(p4f[:, :], pq4[:, :], 128.0)
        tnib = apool.tile([P, HB], f32, name="tnib", tag="tnib")
        nc.vector.tensor_scalar_mul(tnib[:, :], p4f[:, :], 1.0 / 16.0)
        tni = apool.tile([P, HB], i32, name="tni", tag="tni")
        nc.vector.tensor_copy(tni[:, :], tnib[:, :])
        hi4 = apool.tile([P, HB], f32, name="hi4", tag="hi4")
        nc.vector.tensor_copy(hi4[:, :], tni[:, :])
        mfx = apool.tile([P, HB], f32, name="mfx", tag="mfx")
        nc.vector.tensor_tensor(mfx[:, :], tnib[:, :], hi4[:, :], op=op.is_lt)
        nc.vector.tensor_sub(hi4[:, :], hi4[:, :], mfx[:, :])
        lo4 = apool.tile([P, HB], f32, name="lo4", tag="lo4")
        nc.vector.scalar_tensor_tensor(lo4[:, :], hi4[:, :], -16.0, p4f[:, :],
                                       op0=op.mult, op1=op.add)
        pq = apool.tile([P, EMB], f16, name="pq", tag="pq")
        pstr = pq[:, :].ap[0][0]
        nc.vector.tensor_scalar(sap(pq[:, :], 0, [[pstr, P], [2, HB]]),
                                lo4[:, :], rsct[:, blk, 1:2],
                                m8sall[:, blk:blk + 1],
                                op0=op.mult, op1=op.add)
        nc.vector.tensor_scalar(sap(pq[:, :], 1, [[pstr, P], [2, HB]]),
                                hi4[:, :], rsct[:, blk, 1:2],
                                m8sall[:, blk:blk + 1],
                                op0=op.mult, op1=op.add)
        qb = apool.tile([P, EMB], f16, name="qb", tag="qb")
        nc.vector.tensor_add(qb[:, :], fq[:, :], pq[:, :])

        qtp = ps_tr.tile([P, 2, P], f16, name="qtp", tag="tr")
        nc.tensor.transpose(qtp[:, 0, :], qb[:, 0:P], idf16[:, :])
        nc.tensor.transpose(qtp[:, 1, :], qb[:, P:EMB], idf16[:, :])
        qts = apool.tile([P, 2, P], f16, name="qts", tag="qts", bufs=2)
        nc.vector.tensor_copy(qts[:, :, :], qtp[:, :, :])

        offp = ps_mm.tile([P, EMB], f32, name="offp", tag="mm")
        mm(offp, [(qts[:, 0, :], Woff[:, 0, :]), (qts[:, 1, :], Woff[:, 1, :])],
           bias=boff[:1, :])
        off = wpool.tile([P, EMB], f32, name="off", tag="off")
        nc.vector.tensor_copy(off[:, :], offp[:, :])

        attp = ps_sm.tile([P, NH * 16], f32, name="attp", tag="sm")
        mm(attp, [(qts[:, 0, :], Watt[:, 0, :]), (qts[:, 1, :], Watt[:, 1, :])],
           bias=batt[:1, :])
        att = wpool.tile([P, NH, 16], f32, name="att", tag="att")
        nc.vector.tensor_copy(att[:, :, :], attp[:, :].rearrange(
            "p (h l) -> p h l", h=NH))

        # softmax over (l,pt) per head
        rmax = wpool.tile([P, NH], f32, name="rmax", tag="rmax")
        nc.vector.reduce_max(rmax[:, :], att[:, :, :], axis=AX.X)
        exv = wpool.tile([P, NH, 16], f32, name="exv", tag="exv")
        rmaxa = rmax[:, :]
        nc.vector.tensor_sub(exv[:, :, :], att[:, :, :],
                             sap(rmaxa, 0, [rmaxa.ap[0], [1, NH], [0, 16]]))
        nc.scalar.activation(exv[:, :, :], exv[:, :, :], act_f.Exp)
        ssum = wpool.tile([P, NH], f32, name="ssum", tag="ssum")
        nc.vector.reduce_sum(ssum[:, :], exv[:, :, :], axis=AX.X)
        rsum = wpool.tile([P, NH], f32, name="rsum", tag="rsum")
        nc.vector.reciprocal(rsum[:, :], ssum[:, :])
        aw = wpool.tile([P, NH, 16], f32, name="aw", tag="aw")
        rsuma = rsum[:, :]
        nc.vector.tensor_mul(aw[:, :, :], exv[:, :, :],
                             sap(rsuma, 0, [rsuma.ap[0], [1, NH], [0, 16]]))

        def wt(name):
            return wpool.tile([P, EMB], f32, name=name, tag=name)

        loc = wt("loc")
        nc.vector.tensor_mul(loc[:, :], off[:, :], c_invn[:, :])
        refa = refr[:, blk, :]
        for xy in (0, 1):
            lvh = sap(loc[:, :], xy, [loc[:, :].ap[0], [32, NH], [8, NL],
                                      [2, NPT]])
            nc.vector.tensor_add(lvh, lvh,
                                 sap(refa, xy, [refa.ap[0], [0, NH], [2, NL],
                                                [0, NPT]]))
        pix = wt("pix")
        nc.vector.tensor_mul(pix[:, :], loc[:, :], c_pixs[:, :])
        nc.vector.tensor_scalar_add(pix[:, :], pix[:, :], -0.5)

        # floor(pix) robust to cast rounding mode
        xi = wpool.tile([P, EMB], i32, name="xi", tag="xi")
        nc.vector.tensor_copy(xi[:, :], pix[:, :])
        base = wt("base")
        nc.vector.tensor_copy(base[:, :], xi[:, :])
        fixm = wt("fixm")
        nc.vector.tensor_tensor(fixm[:, :], pix[:, :], base[:, :], op=op.is_lt)
        nc.vector.tensor_sub(base[:, :], base[:, :], fixm[:, :])
        wfrac = wt("wfrac")
        nc.vector.tensor_sub(wfrac[:, :], pix[:, :], base[:, :])

        basec = wt("basec")
        nc.vector.tensor_scalar_max(basec[:, :], base[:, :], 0.0)
        nc.vector.tensor_tensor(basec[:, :], basec[:, :], c_clip[:, :],
                                op=op.min)

        v0b = wt("v0b")
        nc.vector.tensor_tensor(v0b[:, :], base[:, :], c_vmax[:, :],
                                op=op.is_le)
        vld0 = wt("vld0")
        nc.vector.scalar_tensor_tensor(vld0[:, :], base[:, :], 0.0, v0b[:, :],
                                       op0=op.is_ge, op1=op.mult)
        v1b = wt("v1b")
        nc.vector.tensor_tensor(v1b[:, :], base[:, :], c_clip[:, :],
                                op=op.is_le)
        vld1 = wt("vld1")
        nc.vector.scalar_tensor_tensor(vld1[:, :], base[:, :], -1.0, v1b[:, :],
                                       op0=op.is_ge, op1=op.mult)

        tsh = wt("tsh")
        nc.vector.tensor_sub(tsh[:, :], base[:, :], basec[:, :])
        e0 = wt("e0")
        nc.vector.tensor_scalar(e0[:, :], tsh[:, :], 0.0, None,
                                op0=op.is_equal)
        em1 = wt("em1")
        nc.vector.tensor_scalar(em1[:, :], tsh[:, :], -1.0, None,
                                op0=op.is_equal)
        ep1 = wt("ep1")
        nc.vector.tensor_scalar(ep1[:, :], tsh[:, :], 1.0, None,
                                op0=op.is_equal)

        u0 = wt("u0")
        nc.vector.tensor_scalar(u0[:, :], wfrac[:, :], -1.0, 1.0, op0=op.mult,
                                op1=op.add)
        nc.vector.tensor_mul(u0[:, :], u0[:, :], vld0[:, :])
        u1 = wt("u1")
        nc.vector.tensor_mul(u1[:, :], wfrac[:, :], vld1[:, :])

        a0 = wt("a0")
        nc.vector.tensor_mul(a0[:, :], u0[:, :], e0[:, :])
        t1 = wt("t1")
        nc.vector.tensor_mul(t1[:, :], u1[:, :], em1[:, :])
        nc.vector.tensor_add(a0[:, :], a0[:, :], t1[:, :])
        a1 = wt("a1")
        nc.vector.tensor_mul(a1[:, :], u0[:, :], ep1[:, :])
        nc.vector.tensor_mul(t1[:, :], u1[:, :], e0[:, :])
        nc.vector.tensor_add(a1[:, :], a1[:, :], t1[:, :])

        def ycols(t):
            return sap(t[:, :], 1, [[t[:, :].ap[0][0], P], [2, P]])

        def xcols(t):
            return sap(t[:, :], 0, [[t[:, :].ap[0][0], P], [2, P]])

        awf = aw.rearrange("p h l -> p (h l)")
        ay0 = wpool.tile([P, P], f32, name="ay0", tag="ay0")
        nc.vector.tensor_mul(ay0[:, :], ycols(a0), awf)
        ay1 = wpool.tile([P, P], f32, name="ay1", tag="ay1")
        nc.vector.tensor_mul(ay1[:, :], ycols(a1), awf)

        w4 = wpool.tile([P, P, 4], f16, name="w4", tag="w4", bufs=2)
        nc.vector.tensor_mul(w4[:, :, 0], ay0[:, :], xcols(a0))
        nc.vector.tensor_mul(w4[:, :, 1], ay0[:, :], xcols(a1))
        nc.vector.tensor_mul(w4[:, :, 2], ay1[:, :], xcols(a0))
        nc.vector.tensor_mul(w4[:, :, 3], ay1[:, :], xcols(a1))

        cell = wpool.tile([P, P], f32, name="cell", tag="cell")
        nc.vector.tensor_mul(cell[:, :], ycols(basec), c_W[:, :])
        nc.vector.tensor_add(cell[:, :], cell[:, :], xcols(basec))
        nc.vector.tensor_add(cell[:, :], cell[:, :], c_S[:, :])

        if gather_mode == "dgather":
            # i16 cell indices rearranged into the SWDGE wrap-16 layout:
            # gather i consumes idxs[i%16, i//16]; we need i = lp*128 + q,
            # so IDX[q%16, h*128 + lp*8 + q//16] = cell(q, h*16+lp)
            celli = wpool.tile([P, P], i16, name="celli", tag="celli")
            nc.vector.tensor_copy(celli[:, :], cell[:, :])
            dma(dap(idxscr, 0, [[1, 8], [NH * P, 16], [P, NH], [8, 16]]),
                celli[:, :])
            idx16 = apool.tile([P, NH * P], i16, name="idx16", tag="idx16",
                               bufs=2)
            dma(idx16, dap(idxscr, 0, [[0, 8], [NH * P, 16], [1, NH * P]]))
            return fq, w4, idx16

        nc.vector.tensor_add(cell[:, :], cell[:, :], c_HL[:, :])
        offs = wpool.tile([P, P], i32, name="offs", tag="offs", bufs=2)
        nc.vector.tensor_copy(offs[:, :], cell[:, :])
        return fq, w4, offs

    # ==================== LayerNorm ====================
    def emit_ln(r, gt, bt, pfx):
        nsum = opool.tile([P, 1], f32, name=f"{pfx}ns", tag=f"{pfx}ns")
        nc.vector.tensor_reduce(nsum[:, :], r[:, :], axis=AX.X, op=op.add,
                                negate=True)
        nmean = opool.tile([P, 1], f32, name=f"{pfx}nm", tag=f"{pfx}nm")
        nc.scalar.mul(nmean[:, :], nsum[:, :], 1.0 / EMB)
        c = opool.tile([P, EMB], f32, name=f"{pfx}c", tag=f"{pfx}c")
        nc.vector.tensor_scalar_add(c[:, :], r[:, :], nmean[:, :])
        csq = opool.tile([P, EMB], f32, name=f"{pfx}sq", tag=f"{pfx}sq")
        ssq = opool.tile([P, 1], f32, name=f"{pfx}ssq", tag=f"{pfx}ssq")
        nc.scalar.activation(csq[:, :], c[:, :], act_f.Square,
                             accum_out=ssq[:, :])
        std = opool.tile([P, 1], f32, name=f"{pfx}std", tag=f"{pfx}std")
        nc.scalar.activation(std[:, :], ssq[:, :], act_f.Sqrt,
                             bias=eps_t[:, :], scale=1.0 / EMB)
        rstd = opool.tile([P, 1], f32, name=f"{pfx}rs", tag=f"{pfx}rs")
        nc.vector.reciprocal(rstd[:, :], std[:, :])
        x = opool.tile([P, EMB], f32, name=f"{pfx}x", tag=f"{pfx}x")
        nc.vector.scalar_tensor_tensor(x[:, :], c[:, :], rstd[:, :], gt[:, :],
                                       op0=op.mult, op1=op.mult)
        nc.vector.tensor_add(x[:, :], x[:, :], bt[:, :])
        return x

    # ==================== per-group pipeline ====================
    def emit_group(g):
        blk = g
        fq, w4, offs = emit_frontend(blk)
        gb = gpool.tile([P, P, 4 * HD], f16, name="gb", tag="gb", bufs=2)
        if gather_mode == "dgather":
            for h in range(NH):
                nc.gpsimd.dma_gather(
                    gb[:, h * 16:(h + 1) * 16, :],
                    dap(tableT, h * L * (4 * HD), [[4 * HD, L], [1, 4 * HD]]),
                    offs[:, h * P:(h + 1) * P],
                    2048, 2048, 4 * HD, single_packet=False)
        elif gather_mode == "batched":
            nc.gpsimd.indirect_dma_start(
                out=gb[:, :, :], out_offset=None,
                in_=tableT.ap()[:, :],
                in_offset=bass.IndirectOffsetOnAxis(ap=offs[:, :], axis=0))
        elif gather_mode.startswith("batched"):
            S = int(gather_mode[len("batched"):])
            for c in range(0, P, S):
                nc.gpsimd.indirect_dma_start(
                    out=gb[:, c:c + S, :], out_offset=None,
                    in_=tableT.ap()[:, :],
                    in_offset=bass.IndirectOffsetOnAxis(ap=offs[:, c:c + S],
                                                        axis=0))
        elif gather_mode == "loop":
            for s in range(P):
                nc.gpsimd.indirect_dma_start(
                    out=gb[:, s, :], out_offset=None,
                    in_=tableT.ap()[:, :],
                    in_offset=bass.IndirectOffsetOnAxis(ap=offs[:, s:s + 1],
                                                        axis=0))
        # gather_mode == "skip": timing-ablation only, gb stays uninitialized

        acat = kpool.tile([P, EMB], f32, name="acat", tag="acat")
        # all-heads combine, reduction tree folded in place inside gb
        gba = gb[:, :, :]
        pstr = gba.ap[0][0]

        def gsl(off, dims):
            return sap(gba, off, [[pstr, P]] + dims)

        # weights: w4 [P, (h,lp), 4] broadcast over head_dim (0-stride)
        w4b = sap(w4[:, :, :], 0,
                  [[w4[:, :, :].ap[0][0], P], [4, P], [1, 4], [0, HD]])
        gall = gsl(0, [[128, P], [HD, 4], [1, HD]])
        nc.vector.tensor_mul(gall, gall, w4b)
        # corner folds: c0+=c1, c2+=c3, c0+=c2
        d2 = [[128, P], [1, HD]]
        nc.vector.tensor_add(gsl(0, d2), gsl(0, d2), gsl(HD, d2))
        nc.vector.tensor_add(gsl(2 * HD, d2), gsl(2 * HD, d2), gsl(3 * HD, d2))
        nc.vector.tensor_add(gsl(0, d2), gsl(0, d2), gsl(2 * HD, d2))
        # lp folds: 16 -> 8 -> 4 -> 2 (per head; h stride 16*128)
        for w in (8, 4, 2):
            dh = [[16 * 128, NH], [128, w], [1, HD]]
            nc.vector.tensor_add(gsl(0, dh), gsl(0, dh), gsl(w * 128, dh))
        # final fold writes the fp32 attention output slice layout
        acv = sap(acat[:, :], 0, [[acat[:, :].ap[0][0], P], [HD, NH], [1, HD]])
        dh1 = [[16 * 128, NH], [1, HD]]
        nc.vector.tensor_add(acv, gsl(0, dh1), gsl(128, dh1))

        # ---- output projection + LN + FFN + LN ----
        ac16 = opool.tile([P, EMB], f16, name="ac16", tag="ac16")
        nc.vector.tensor_copy(ac16[:, :], acat[:, :])
        atp = ps_tr.tile([P, 2, P], f16, name="atp", tag="tr")
        nc.tensor.transpose(atp[:, 0, :], ac16[:, 0:P], idf16[:, :])
        nc.tensor.transpose(atp[:, 1, :], ac16[:, P:EMB], idf16[:, :])
        ats = opool.tile([P, 2, P], f16, name="ats", tag="ats")
        nc.vector.tensor_copy(ats[:, :, :], atp[:, :, :])
        oprj = ps_mm.tile([P, EMB], f32, name="oprj", tag="mm")
        mm(oprj, [(ats[:, 0, :], Wout[:, 0, :]),
                  (ats[:, 1, :], Wout[:, 1, :])], bias=bout[:1, :])

        r1 = opool.tile([P, EMB], f32, name="r1", tag="r1")
        nc.vector.tensor_add(r1[:, :], oprj[:, :], fq[:, :])
        x1 = emit_ln(r1, ln1g, ln1b, "la")

        x16 = opool.tile([P, EMB], f16, name="x16", tag="x16")
        nc.vector.tensor_copy(x16[:, :], x1[:, :])
        xtp = ps_tr.tile([P, 2, P], f16, name="xtp", tag="tr")
        nc.tensor.transpose(xtp[:, 0, :], x16[:, 0:P], idf16[:, :])
        nc.tensor.transpose(xtp[:, 1, :], x16[:, P:EMB], idf16[:, :])
        xts = opool.tile([P, 2, P], f16, name="xts", tag="xts")
        nc.vector.tensor_copy(xts[:, :, :], xtp[:, :, :])

        h1s = opool.tile([P, DFFN // P, P], f16, name="h1s", tag="h1s")
        hp = ps_mm.tile([P, DFFN // P, P], f32, name="hp", tag="hpw", bufs=1)
        for mt in range(DFFN // P):
            nc.tensor.matmul(hp[:, mt, :], W1[:, 0, mt * P:(mt + 1) * P],
                             xts[:, 0, :], start=True, stop=False)
            nc.tensor.matmul(hp[:, mt, :], W1[:, 1, mt * P:(mt + 1) * P],
                             xts[:, 1, :], start=False, stop=False)
            nc.tensor.matmul(hp[:, mt, :], b1r[:1, mt * P:(mt + 1) * P],
                             onesr[:1, :], start=False, stop=True)
        nc.scalar.activation(h1s[:, :, :], hp[:, :, :], act_f.Relu)

        yp = ps_mm.tile([P, EMB], f32, name="yp", tag="mm")
        for mt in range(DFFN // P):
            nc.tensor.matmul(yp[:, :], h1s[:, mt, :], W2[:, mt, :],
                             start=(mt == 0), stop=False)
        nc.tensor.matmul(yp[:, :], onesr[:1, :], b2r[:1, :],
                         start=False, stop=True)

        r2 = opool.tile([P, EMB], f32, name="r2", tag="r2")
        nc.vector.tensor_add(r2[:, :], yp[:, :], x1[:, :])
        x2 = emit_ln(r2, ln2g, ln2b, "lb")

        # per-row int8 quantization: scale = rowmax/127, shipped alongside
        absx = opool.tile([P, EMB], f32, name="absx", tag="absx")
        nc.scalar.activation(absx[:, :], x2[:, :], act_f.Abs)
        rmax = opool.tile([P, 1], f32, name="rmax2", tag="rmax2")
        nc.vector.reduce_max(rmax[:, :], absx[:, :], axis=AX.X)
        nc.vector.tensor_scalar_max(rmax[:, :], rmax[:, :], 1e-6)
        rinv = opool.tile([P, 1], f32, name="rinv", tag="rinv")
        nc.vector.reciprocal(rinv[:, :], rmax[:, :])
        smul = opool.tile([P, 1], f32, name="smul", tag="smul")
        nc.scalar.mul(smul[:, :], rinv[:, :], 127.0)
        osc = opool.tile([P, 1], f16, name="osc", tag="osc")
        nc.scalar.mul(osc[:, :], rmax[:, :], 1.0 / 127.0)
        # round(x*smul) = floor(x*smul + 0.5), floor robust to cast mode
        tq = opool.tile([P, EMB], f32, name="tq", tag="tq")
        nc.vector.tensor_scalar(tq[:, :], x2[:, :], smul[:, :], 0.5,
                                op0=op.mult, op1=op.add)
        qi = opool.tile([P, EMB], i32, name="qi", tag="qi")
        nc.vector.tensor_copy(qi[:, :], tq[:, :])
        qf = opool.tile([P, EMB], f32, name="qf", tag="qf")
        nc.vector.tensor_copy(qf[:, :], qi[:, :])
        qm = opool.tile([P, EMB], f32, name="qm", tag="qm")
        nc.vector.tensor_tensor(qm[:, :], tq[:, :], qf[:, :], op=op.is_lt)
        nc.vector.tensor_sub(qf[:, :], qf[:, :], qm[:, :])
        x2q = opool.tile([P, EMB], i8, name="x2q", tag="x2q")
        nc.vector.tensor_copy(x2q[:, :], qf[:, :])
        dma(outs["out_q"][blk * P:(blk + 1) * P, :], x2q)
        dma(outs["out_s"][blk * P:(blk + 1) * P, :], osc)

    for g in range(NGRP):
        emit_group(g)

    ctx.close()


# ------------------------------------------------------------ host entry ---

_CACHE = {}


def build_nc(cfg, gather_mode="dgather", dyn_scratch=16384):
    import concourse.bass as bass
    from concourse import bacc, mybir, tile

    nc = bacc.Bacc("TRN2", debug=False, num_devices=8,
                   dynamic_dma_scratch_size=dyn_scratch)
    f32 = mybir.dt.float32
    f16 = mybir.dt.float16

    def di(name, shape, dt=None):
        return nc.dram_tensor(name, list(shape), dt or f32,
                              kind="ExternalInput").ap()

    i8 = mybir.dt.int8
    HQ = cfg["HQ"]
    ins = dict(
        feat_h=di("feat_h", [HQ, EMB], i8),
        pos_h=di("pos_h", [HQ, EMB // 2], i8),
        rsc=di("rsc", [HQ, 2]),
        ref_q=di("ref_q", [HQ, NL, 2], f16),
        wchunk=di("wchunk", [WCHUNK], f16),
        sb16=di("sb16", [1, SB16TOT], f16),
        sb32=di("sb32", [1, SB32TOT]),
    )
    outs = dict(
        out_q=nc.dram_tensor("out_q", [HQ, EMB], i8,
                             kind="ExternalOutput").ap(),
        out_s=nc.dram_tensor("out_s", [HQ, 1], f16,
                             kind="ExternalOutput").ap(),
    )
    with tile.TileContext(nc) as tc:
        emit_kernel(tc, outs, ins, cfg, gather_mode)
    nc.compile()
    return nc


def make_in_maps(inputs, cfg):
    feats = np.asarray(inputs["features"], np.float32)
    pos = np.asarray(inputs["pos"], np.float32)
    refp = np.asarray(inputs["reference_points"], np.float32)
    B = feats.shape[0]
    HQ, L = cfg["HQ"], cfg["L"]
    half = L // 2

    consts = host_constants(cfg)
    wblob = np.concatenate(
        [np.asarray(inputs[k], np.float32).astype(np.float16).reshape(-1)
         for k in WORDER])
    assert wblob.size == WTOT

    sb16src = dict(b_val=inputs["b_val"], b_off=inputs["b_off"],
                   b_attn=inputs["b_attn"], b_out=inputs["b_out"],
                   b1=inputs["b1"], b2=inputs["b2"],
                   ones_row=consts["ones_row"], ident=consts["ident"])
    sb16 = np.concatenate(
        [np.asarray(sb16src[n], np.float32).reshape(-1)
         for n, _ in SB16ORD]).astype(np.float16).reshape(1, -1)
    assert sb16.size == SB16TOT
    sb32src = dict(ln1_g=inputs["ln1_g"], ln1_b=inputs["ln1_b"],
                   ln2_g=inputs["ln2_g"], ln2_b=inputs["ln2_b"],
                   cst_xy=consts["cst_xy"], cst_hlp=consts["cst_hlp"])
    sb32 = np.concatenate(
        [np.asarray(sb32src[n], np.float32).reshape(-1)
         for n, _ in SB32ORD]).astype(np.float32).reshape(1, -1)
    assert sb32.size == SB32TOT

    def quant_rows(x, nrow):
        # per-row symmetric int8: q = round(x/scale), scale = rowmax/127
        q = np.zeros((nrow, x.shape[1]), np.int8)
        sc = np.ones((nrow, 1), np.float32)
        mx = np.abs(x).max(axis=1, keepdims=True)
        mx = np.maximum(mx, 1e-12)
        sc[:x.shape[0]] = (mx / 127.0).astype(np.float32)
        q[:x.shape[0]] = np.clip(np.rint(x / (mx / 127.0)), -127, 127
                                 ).astype(np.int8)
        return q, sc

    def quant_pos4(x, nrow):
        # per-row int4 nibbles: v = clip(round(x/s), -7, 7)+8, s = rowmax/7;
        # packed pairs (lo | hi<<4) shipped as biased int8 (byte - 128)
        pk = np.zeros((nrow, x.shape[1] // 2), np.int8)
        sc = np.ones((nrow, 1), np.float32)
        mx = np.maximum(np.abs(x).max(axis=1, keepdims=True), 1e-12)
        s = (mx / 7.0).astype(np.float32)
        q = (np.clip(np.rint(x / s), -7, 7) + 8).astype(np.int16)
        by = q[:, 0::2] + (q[:, 1::2] << 4)
        pk[:x.shape[0]] = (by - 128).astype(np.int8)
        pk[x.shape[0]:] = 8 + (8 << 4) - 128
        sc[:x.shape[0]] = s
        return pk, sc

    halves = [(0, half), (half, L)]
    in_maps = []
    for core in range(2 * B):
        b, hf = core // 2, core % 2
        s, e = halves[hf]
        fh, fsc = quant_rows(feats[b, s:e], HQ)
        ph, psc = quant_pos4(pos[b, s:e], HQ)
        rq = np.zeros((HQ, NL, 2), np.float16)
        rq[:e - s] = refp[b, s:e].astype(np.float16)
        m = dict(feat_h=fh, pos_h=ph, ref_q=rq,
                 rsc=np.ascontiguousarray(np.hstack([fsc, psc])),
                 wchunk=np.ascontiguousarray(
                     wblob[core * WCHUNK:(core + 1) * WCHUNK]),
                 sb16=sb16, sb32=sb32)
        in_maps.append(m)
    return in_maps, halves


def assemble_out(res, B, L, halves):
    out = np.zeros((B, L, EMB), np.float32)
    for core in range(2 * B):
        b, hf = core // 2, core % 2
        s, e = halves[hf]
        n = e - s
        q = res.results[core]["out_q"][:n].astype(np.float32)
        sc = res.results[core]["out_s"][:n]
        out[b, s:e] = q * sc
    return out


def kernel(**inputs):
    from concourse import bass_utils

    cfg = CFG_FULL
    in_maps, halves = make_in_maps(inputs, cfg)
    B = np.asarray(inputs["features"]).shape[0]
    L = cfg["L"]

    if "nc" not in _CACHE:
        _CACHE["nc"] = build_nc(cfg)
    nc = _CACHE["nc"]

    res = bass_utils.run_bass_kernel_spmd(nc, in_maps,
                                          core_ids=list(range(2 * B)))
    return assemble_out(res, B, L, halves)
